# revision 15
# baseline (speedup 1.0000x reference)
"""Trainium2 Bass kernel for nn_PolicyNetwork3 (2-layer GraphSAGE + edge-MLP).

v2 design (8 NeuronCores, SPMD single NEFF):
- dst-sharded aggregation; core k owns node block [6272k, 6272k+6272).
- Edge messages gathered per-edge from bf16 HBM row tables via 4-queue
  round-robin dma_gather (descriptor generation parallelizes across the
  SWDGE queues; ~2.4ns/row vs 8ns serialized).
- segment-sum per 256-dst window via one-hot matmuls (bf16 gathered rows x
  fp8 0/1 one-hot streamed from HBM) accumulating in PSUM across the LO/HI
  half-table runs; per-window PSUM->SBUF copy fuses the 1/deg scale.
- BN folded into SAGE weights on host; linear+leaky per 128-node block;
  h shards exchanged with bf16 AllGather.
- candidate MLP: g/q tables per node; transposed candidate gathers put
  features on partitions so the whole MLP runs as 512-wide PE matmuls
  (identity-matmul adds, K=1 bias rows); global softmax on-device.
"""

import os
import sys

sys.path.insert(0, "/opt/trn_rl_repo")
sys.path.insert(0, "/root/.axon_site")

import numpy as np
import ml_dtypes

import concourse.bacc as bacc
import concourse.bass as bass
import concourse.bass_isa as bass_isa
import concourse.mybir as mybir
import concourse.tile as tile
from concourse import library_config
from concourse.bass_utils import run_bass_kernel_spmd

P = 128
N, E, C = 50000, 800000, 100000
D = 128
NCORE = 8
NSH = 6272            # nodes per core shard
NTOT = NSH * NCORE    # 50176 padded node table
HALF = NTOT // 2      # 25088 rows per gather-table half
PART0 = 3200          # local rows in table part 0 (25 blocks)
PART1 = NSH - PART0   # 3072 local rows in part 1
T0 = NCORE * PART0    # 25600 rows in part-0 table
T1 = NCORE * PART1    # 24576 rows in part-1 table
W = 256               # dst window width (PSUM cols)
NWIN = (NSH + W - 1) // W   # 25 windows (last one 128 wide)
NBLK = NSH // P       # 49 node blocks for the linear phase
CSH = C // NCORE      # 12500 candidates per core
GCALL = 2048          # max idxs per dma_gather call
CCH = 512             # candidate MLP chunk
BN_EPS = 1e-5
SLOPE = 0.01
F32 = mybir.dt.float32
BF16 = mybir.dt.bfloat16
F8 = mybir.dt.float8e4
I16 = mybir.dt.int16
AF = mybir.ActivationFunctionType
ALU = mybir.AluOpType
BF = ml_dtypes.bfloat16
F8NP = ml_dtypes.float8_e4m3fn


def _wrap16(idx_lin):
    """[n] -> [128, n/16] int16 in the dma_gather wrapped+replicated layout."""
    n = idx_lin.shape[0]
    assert n % 16 == 0
    w = idx_lin.reshape(n // 16, 16).T.astype(np.int16)
    return np.tile(w, (8, 1)).copy()


def _win_width(w):
    return min(W, NSH - w * W)


def _node_part(g):
    """Global node id -> (part, int16 table index) under the shard-half
    permuted table layout [part0: 8 x rows 0:3200 | part1: 8 x rows 3200:]."""
    k, r = g // NSH, g % NSH
    p = (r >= PART0).astype(np.int64)
    idx = np.where(p == 0, k * PART0 + r, k * PART1 + (r - PART0))
    return p, idx


def _prep_edges(src, dst):
    """Uniform per-core window/run/chunk schedule + per-core idx and one-hot."""
    core = dst // NSH
    winl = (dst - core * NSH) // W
    dstloc = (dst - core * NSH) - winl * W
    half, tidx = _node_part(src)
    key = (core * NWIN + winl) * 2 + half
    order = np.argsort(key, kind="stable")
    cnt = np.bincount(key, minlength=NCORE * NWIN * 2).reshape(NCORE, NWIN, 2)
    nch_u = (-(-cnt // P)).max(axis=0)        # [NWIN, 2] uniform chunk counts
    # global chunk order: w0 LO chunks, w0 HI, w1 LO, ...
    win_ch0 = np.zeros(NWIN + 1, np.int64)
    np.cumsum(nch_u.sum(axis=1), out=win_ch0[1:])
    totch = int(win_ch0[-1])
    nslot = totch * P
    # gather calls (uniform): per (w, half) run split into balanced pieces
    # of ~CTGT chunks so the 4 SWDGE queues stay evenly loaded
    CTGT = 8
    calls = {}  # (w, s) -> list of (slot_start, n_idx, chunk0)
    for w in range(NWIN):
        c0 = int(win_ch0[w])
        for s in (0, 1):
            nch = int(nch_u[w, s])
            cl = []
            if nch > 0:
                npiece = -(-nch // CTGT)
                base, rem = divmod(nch, npiece)
                cc = c0
                for i in range(npiece):
                    sz = base + (1 if i < rem else 0)
                    cl.append((cc * P, sz * P, cc))
                    cc += sz
            calls[(w, s)] = cl
            c0 += nch
    bstart = np.zeros(NCORE * NWIN * 2 + 1, np.int64)
    np.cumsum(np.bincount(key, minlength=NCORE * NWIN * 2), out=bstart[1:])
    gidx = np.zeros((NCORE, nslot), np.int16)
    ohm = np.zeros((NCORE, P, totch * W), np.uint8)  # fp8 bits (1.0 = 0x38)
    ONE = np.float32(1.0).astype(F8NP).view(np.uint8)
    for k in range(NCORE):
        for w in range(NWIN):
            pos = int(win_ch0[w]) * P
            for s in (0, 1):
                b = (k * NWIN + w) * 2 + s
                sl = order[bstart[b]:bstart[b + 1]]
                n = len(sl)
                gidx[k, pos:pos + n] = tidx[sl].astype(np.int16)
                slots = pos + np.arange(n)
                ohm[k, slots % P, (slots // P) * W + dstloc[sl]] = ONE
                pos += int(nch_u[w, s]) * P
    meta = dict(nch_u=nch_u, win_ch0=win_ch0, totch=totch, nslot=nslot,
                calls=calls)
    data = [dict(gidx=_wrap16(gidx[k]), ohm=ohm[k].view(F8NP)) for k in range(NCORE)]
    return meta, data


def _prep_cands(cand_u, cand_v, cand_feat):
    """Shard candidates, group by (u_half, v_half), pad groups to 128."""
    gch = np.zeros((NCORE, 4), np.int64)
    groups = [[None] * 4 for _ in range(NCORE)]
    for k in range(NCORE):
        ids = np.arange(k * CSH, (k + 1) * CSH)
        pu, _ = _node_part(cand_u[ids])
        pv, _ = _node_part(cand_v[ids])
        g = pu * 2 + pv
        for gi in range(4):
            groups[k][gi] = ids[g == gi]
            gch[k, gi] = -(-len(groups[k][gi]) // P)
    gch_u = gch.max(axis=0)
    goff = np.zeros(5, np.int64)
    np.cumsum(gch_u * P, out=goff[1:])
    cslot = int(goff[4])                       # 128-granular
    cu = np.zeros((NCORE, cslot), np.int16)
    cv = np.zeros((NCORE, cslot), np.int16)
    ft = np.zeros((NCORE, cslot), BF)
    mask = np.full((NCORE, cslot), -1e30, np.float32).astype(BF)
    slotmap = np.full((NCORE, cslot), -1, np.int64)
    for k in range(NCORE):
        for gi in range(4):
            ids = groups[k][gi]
            n = len(ids)
            p0 = int(goff[gi])
            _, uix = _node_part(cand_u[ids])
            _, vix = _node_part(cand_v[ids])
            cu[k, p0:p0 + n] = uix.astype(np.int16)
            cv[k, p0:p0 + n] = vix.astype(np.int16)
            ft[k, p0:p0 + n] = cand_feat[ids, 0].astype(BF)
            mask[k, p0:p0 + n] = 0.0
            slotmap[k, p0:p0 + n] = ids
    # u calls: groups 0-1 (uh=0) then 2-3 (uh=1); v calls per group;
    # balanced ~8-chunk pieces
    CTGT = 8

    def _split(lo, hi, s, out):
        nch = (hi - lo) // P
        if nch <= 0:
            return
        npiece = -(-nch // CTGT)
        base, rem = divmod(nch, npiece)
        p = lo
        for i in range(npiece):
            sz = (base + (1 if i < rem else 0)) * P
            out.append((p, sz, s))
            p += sz

    ucalls, vcalls = [], []
    _split(0, int(goff[2]), 0, ucalls)
    _split(int(goff[2]), int(goff[4]), 1, ucalls)
    for gi in range(4):
        _split(int(goff[gi]), int(goff[gi + 1]), gi % 2, vcalls)
    meta = dict(cslot=cslot, ucalls=ucalls, vcalls=vcalls)
    data = [dict(cu=_wrap16(cu[k]), cv=_wrap16(cv[k]), feat=ft[k][None, :],
                 mask=mask[k][None, :], slotmap=slotmap[k]) for k in range(NCORE)]
    return meta, data


def _build_nc(em, cm):
    nc = bacc.Bacc("TRN2", target_bir_lowering=False, debug=False,
                   num_devices=NCORE, num_swdge_queues=4)
    TOTCH, NSLOT = em["totch"], em["nslot"]
    CSLOT = cm["cslot"]
    NCC = -(-CSLOT // CCH)                  # candidate MLP chunk groups
    YCOLS = NCORE * CSLOT // P              # yfull viewed as [128, YCOLS]

    xb = nc.dram_tensor("xb", [NTOT, D], BF16, kind="ExternalInput")
    xT = nc.dram_tensor("xT", [P, NSH], BF16, kind="ExternalInput")
    gidx = nc.dram_tensor("gidx", [P, NSLOT // 16], I16, kind="ExternalInput")
    ohm = nc.dram_tensor("ohm", [P, TOTCH * W], F8, kind="ExternalInput")
    invd = nc.dram_tensor("invd", [P, NSH], BF16, kind="ExternalInput")
    wself = [nc.dram_tensor(f"wself{l}", [D, D], BF16, kind="ExternalInput") for l in range(2)]
    wneigh = [nc.dram_tensor(f"wneigh{l}", [D, D], BF16, kind="ExternalInput") for l in range(2)]
    crow = [nc.dram_tensor(f"crow{l}", [1, D], BF16, kind="ExternalInput") for l in range(2)]
    identb = nc.dram_tensor("identb", [P, P], BF16, kind="ExternalInput")
    onesr = nc.dram_tensor("onesr", [1, P], BF16, kind="ExternalInput")
    onesc = nc.dram_tensor("onesc", [1, CCH], BF16, kind="ExternalInput")
    amat = nc.dram_tensor("amat", [D, 64], BF16, kind="ExternalInput")
    bmat = nc.dram_tensor("bmat", [D, 64], BF16, kind="ExternalInput")
    mw0r = nc.dram_tensor("mw0r", [1, 64], BF16, kind="ExternalInput")
    mb0r = nc.dram_tensor("mb0r", [1, 64], BF16, kind="ExternalInput")
    mw1 = nc.dram_tensor("mw1", [64, 64], BF16, kind="ExternalInput")
    mb1r = nc.dram_tensor("mb1r", [1, 64], BF16, kind="ExternalInput")
    mw2 = nc.dram_tensor("mw2c", [64, 1], BF16, kind="ExternalInput")
    mb2r = nc.dram_tensor("mb2r", [1, 1], BF16, kind="ExternalInput")
    cu = nc.dram_tensor("cu", [P, CSLOT // 16], I16, kind="ExternalInput")
    cv = nc.dram_tensor("cv", [P, CSLOT // 16], I16, kind="ExternalInput")
    featr = nc.dram_tensor("featr", [1, CSLOT], BF16, kind="ExternalInput")
    maskr = nc.dram_tensor("maskr", [1, CSLOT], BF16, kind="ExternalInput")

    y_out = nc.dram_tensor("y_out", [P, YCOLS], F32, kind="ExternalOutput")
    p_out = nc.dram_tensor("p_out", [P, YCOLS], F32, kind="ExternalOutput")

    hshA = nc.dram_tensor("hshA", [PART0, D], BF16, kind="Internal")
    hshB = nc.dram_tensor("hshB", [PART1, D], BF16, kind="Internal")
    hfull0 = nc.dram_tensor("hfull0", [T0, D], BF16, kind="Internal",
                            addr_space="Shared")
    hfull1 = nc.dram_tensor("hfull1", [T1, D], BF16, kind="Internal",
                            addr_space="Shared")
    gqshA = nc.dram_tensor("gqshA", [PART0, D], BF16, kind="Internal")
    gqshB = nc.dram_tensor("gqshB", [PART1, D], BF16, kind="Internal")
    gqfull0 = nc.dram_tensor("gqfull0", [T0, D], BF16, kind="Internal",
                             addr_space="Shared")
    gqfull1 = nc.dram_tensor("gqfull1", [T1, D], BF16, kind="Internal",
                             addr_space="Shared")
    ysh = nc.dram_tensor("ysh", [1, CSLOT], F32, kind="Internal")
    yfull = nc.dram_tensor("yfull", [NCORE, CSLOT], F32, kind="Internal",
                           addr_space="Shared")

    rg = [list(range(NCORE))]
    nch_u, win_ch0, calls = em["nch_u"], em["win_ch0"], em["calls"]

    with tile.TileContext(nc) as tc:
        with (
            tc.tile_pool(name="const", bufs=1) as cp,
            tc.tile_pool(name="big", bufs=1) as bp,
            tc.tile_pool(name="msgs", bufs=9) as mp,
            tc.tile_pool(name="ohp", bufs=4) as op_,
            tc.tile_pool(name="wrk", bufs=4) as wp,
            tc.tile_pool(name="frows", bufs=3) as fp_,
            tc.tile_pool(name="gup", bufs=3) as gup,
            tc.tile_pool(name="gvp", bufs=3) as gvp,
            tc.tile_pool(name="z1p", bufs=2) as z1p,
        ):
            nc.gpsimd.load_library(library_config.mlp)

            def load(pool, t, shape=None):
                tl = pool.tile(shape or list(t.shape), t.dtype, tag=t.name)
                nc.sync.dma_start(tl[:], t[:])
                return tl

            gidx_t = load(cp, gidx)
            invd_t = load(cp, invd)
            identb_t = load(cp, identb)
            onesr_t = load(cp, onesr)
            onesc_t = load(cp, onesc)
            wself_t = [load(cp, t) for t in wself]
            wneigh_t = [load(cp, t) for t in wneigh]
            crow_t = [load(cp, t) for t in crow]
            amat_t = load(cp, amat)
            bmat_t = load(cp, bmat)
            mw0r_t = load(cp, mw0r)
            mb0r_t = load(cp, mb0r)
            mw1_t = load(cp, mw1)
            mb1r_t = load(cp, mb1r)
            mw2_t = load(cp, mw2)
            mb2r_t = load(cp, mb2r)
            cu_t = load(cp, cu)
            cv_t = load(cp, cv)
            featr_t = load(cp, featr)

            curT = bp.tile([P, NSH], BF16, tag="curT")
            nxtT = bp.tile([P, NSH], BF16, tag="nxtT")
            aggrb = bp.tile([P, NSH], BF16, tag="aggrb")
            nc.sync.dma_start(curT[:], xT[:])

            qrr = [0]

            def next_q():
                q = qrr[0]
                qrr[0] = (q + 1) % 4
                return q

            ctxA = tc.tile_pool(name="ps_w", bufs=2, space="PSUM")
            ps_w = ctxA.__enter__()
            ctxB = tc.tile_pool(name="ps_l", bufs=2, space="PSUM")
            ps_l = ctxB.__enter__()
            ctxC = tc.tile_pool(name="ps_t", bufs=1, space="PSUM")
            ps_t = ctxC.__enter__()

            def linear_block(layer, b):
                ph = ps_l.tile([P, P], F32, tag="ph")
                nc.tensor.matmul(ph[:], lhsT=aggrb[:, b * P:(b + 1) * P],
                                 rhs=wneigh_t[layer][:], start=True, stop=False)
                nc.tensor.matmul(ph[:], lhsT=curT[0][:, b * P:(b + 1) * P],
                                 rhs=wself_t[layer][:], start=False, stop=False)
                nc.tensor.matmul(ph[:], lhsT=onesr_t[:], rhs=crow_t[layer][:],
                                 start=False, stop=True)
                tmp = wp.tile([P, P], BF16, tag="tmp")
                nc.vector.tensor_scalar(tmp[:], ph[:], SLOPE, None, ALU.mult)
                ht = wp.tile([P, P], BF16, tag="ht")
                nc.vector.tensor_tensor(ht[:], ph[:], tmp[:], ALU.max)
                if layer == 0:
                    if b < PART0 // P:
                        nc.sync.dma_start(hshA[b * P:(b + 1) * P, :], ht[:])
                    else:
                        b1 = b - PART0 // P
                        nc.sync.dma_start(hshB[b1 * P:(b1 + 1) * P, :], ht[:])
                pt = ps_t.tile([P, P], BF16, tag="pt")
                nc.tensor.transpose(pt[:], ht[:], identb_t[:])
                nc.scalar.activation(nxtT[0][:, b * P:(b + 1) * P], pt[:],
                                     AF.Copy)
                if layer == 0:
                    if b == PART0 // P - 1:
                        nc.gpsimd.collective_compute(
                            "AllGather", ALU.bypass, replica_groups=rg,
                            ins=[hshA[:].opt()], outs=[hfull0[:].opt()])
                else:
                    pg = ps_l.tile([P, P], F32, tag="ph")
                    nc.tensor.matmul(pg[:, 0:64],
                                     lhsT=nxtT[0][:, b * P:(b + 1) * P],
                                     rhs=amat_t[:], start=True, stop=True)
                    nc.tensor.matmul(pg[:, 64:128],
                                     lhsT=nxtT[0][:, b * P:(b + 1) * P],
                                     rhs=bmat_t[:], start=True, stop=False)
                    nc.tensor.matmul(pg[:, 64:128], lhsT=onesr_t[:],
                                     rhs=mb0r_t[:], start=False, stop=True)
                    gq = wp.tile([P, P], BF16, tag="gq")
                    nc.scalar.activation(gq[:], pg[:], AF.Copy)
                    if b < PART0 // P:
                        nc.sync.dma_start(gqshA[b * P:(b + 1) * P, :], gq[:])
                    else:
                        b1 = b - PART0 // P
                        nc.sync.dma_start(gqshB[b1 * P:(b1 + 1) * P, :], gq[:])
                    if b == PART0 // P - 1:
                        nc.gpsimd.collective_compute(
                            "AllGather", ALU.bypass, replica_groups=rg,
                            ins=[gqshA[:].opt()],
                            outs=[gqfull0[:].opt()])

            def run_gathers(ps, gtab_s, cl, first, mtot):
                done = first
                for (p0, n_idx, ch0) in cl:
                    ncall = n_idx // P
                    g = mp.tile([P, GCALL // P, P], BF16, tag="g")
                    nc.gpsimd.dma_gather(
                        g[:, :ncall, :], gtab_s[:],
                        gidx_t[:, p0 // 16:(p0 + n_idx) // 16],
                        n_idx, n_idx, P, single_packet=False,
                        queue_num=next_q())
                    oht = op_.tile([P, GCALL // P, W], F8, tag="oh")
                    nc.sync.dma_start(
                        oht[:, :ncall, :],
                        ohm[:, ch0 * W:(ch0 + ncall) * W]
                        .rearrange("p (a b) -> p a b", b=W))
                    for cc in range(ncall):
                        nc.tensor.matmul(
                            ps[:], lhsT=g[:, cc, :], rhs=oht[:, cc, :],
                            start=(done == 0), stop=(done == mtot - 1))
                        done += 1

            curT, nxtT = [curT], [nxtT]
            for layer in range(2):
                tabs = ((xb[0:T0, :], xb[T0:NTOT, :]) if layer == 0
                        else (hfull0[:], hfull1[:]))
                # pass 0: LO runs -> PSUM -> aggrb
                for w in range(NWIN):
                    ww = _win_width(w)
                    nch = int(nch_u[w, 0])
                    if nch == 0:
                        nc.vector.memset(aggrb[:, w * W:w * W + ww], 0.0)
                        continue
                    ps = ps_w.tile([P, W], F32, tag="psw")
                    run_gathers(ps, tabs[0], calls[(w, 0)], 0, nch)
                    nc.scalar.activation(aggrb[:, w * W:w * W + ww],
                                         ps[:, :ww], AF.Copy)
                # pass 1: HI runs -> PSUM -> add + 1/deg scale -> linear
                for w in range(NWIN):
                    ww = _win_width(w)
                    wsl = aggrb[:, w * W:w * W + ww]
                    nch = int(nch_u[w, 1])
                    if nch > 0:
                        ps = ps_w.tile([P, W], F32, tag="psw")
                        run_gathers(ps, tabs[1], calls[(w, 1)], 0, nch)
                        nc.vector.tensor_tensor(wsl, wsl, ps[:, :ww], ALU.add)
                    nc.vector.tensor_tensor(wsl, wsl,
                                            invd_t[:, w * W:w * W + ww],
                                            ALU.mult)
                    for b in range((w * W) // P, (w * W + ww) // P):
                        linear_block(layer, b)
                if layer == 0:
                    nc.gpsimd.collective_compute(
                        "AllGather", ALU.bypass, replica_groups=rg,
                        ins=[hshB[:].opt()], outs=[hfull1[:].opt()])
                curT[0], nxtT[0] = nxtT[0], curT[0]
            nc.gpsimd.collective_compute(
                "AllGather", ALU.bypass, replica_groups=rg,
                ins=[gqshB[:].opt()], outs=[gqfull1[:].opt()])

            ctxC.__exit__(None, None, None)
            ctxB.__exit__(None, None, None)
            ctxA.__exit__(None, None, None)
            with (
                tc.tile_pool(name="psc1", bufs=2, space="PSUM") as psc1,
                tc.tile_pool(name="psc2", bufs=2, space="PSUM") as psc2,
                tc.tile_pool(name="ps_c", bufs=2, space="PSUM") as ps_c,
                tc.tile_pool(name="ps_y", bufs=1, space="PSUM") as ps_y,
            ):
                # ---- candidate gathers (slot layout, 4-queue) ----
                ucalls, vcalls = cm["ucalls"], cm["vcalls"]
                uci, vci = [], []
                for i, (s0, n, sh) in enumerate(ucalls):
                    uci += [(i, j) for j in range(n // P)]
                for i, (s0, n, sh) in enumerate(vcalls):
                    vci += [(i, j) for j in range(n // P)]
                gu_tiles = [None] * len(ucalls)
                gv_tiles = [None] * len(vcalls)
                for i in range(max(len(ucalls), len(vcalls))):
                    for (tiles, calls, pool, idx_t) in (
                            (gu_tiles, ucalls, gup, cu_t),
                            (gv_tiles, vcalls, gvp, cv_t)):
                        if i >= len(calls):
                            continue
                        (s0, n_idx, sh) = calls[i]
                        ncall = n_idx // P
                        g = pool.tile([P, GCALL // P, P], BF16, tag="g")
                        nc.gpsimd.dma_gather(
                            g[:, :ncall, :],
                            (gqfull0 if sh == 0 else gqfull1)[:],
                            idx_t[:, s0 // 16:(s0 + n_idx) // 16],
                            n_idx, n_idx, P, single_packet=False,
                            queue_num=next_q())
                        tiles[i] = g
                # ---- candidate MLP ----
                nchunk = CSLOT // P
                z1t = None
                for c in range(nchunk):
                    ui, uo = uci[c]
                    vi, vo = vci[c]
                    psf = psc1.tile([P, 64], F32, tag="psf")
                    nc.tensor.matmul(psf[:], lhsT=identb_t[:],
                                     rhs=gu_tiles[ui][:, uo, 0:64],
                                     start=True, stop=False)
                    nc.tensor.matmul(psf[:], lhsT=identb_t[:],
                                     rhs=gv_tiles[vi][:, vo, 64:128],
                                     start=False, stop=False)
                    nc.tensor.matmul(psf[:], lhsT=featr_t[:, c * P:(c + 1) * P],
                                     rhs=mw0r_t[:], start=False, stop=True)
                    zs = wp.tile([P, 64], BF16, tag="zs")
                    nc.vector.tensor_scalar(zs[:], psf[:], SLOPE, None, ALU.mult)
                    z1b = wp.tile([P, 64], BF16, tag="z1b")
                    nc.vector.tensor_tensor(z1b[:], psf[:], zs[:], ALU.max)
                    ptz = psc2.tile([64, P], BF16, tag="ptz")
                    nc.tensor.transpose(ptz[:], z1b[:], identb_t[:])
                    if c % 4 == 0:
                        z1t = z1p.tile([64, CCH], BF16, tag="z1t")
                    nc.scalar.activation(z1t[:, (c % 4) * P:(c % 4 + 1) * P],
                                         ptz[:], AF.Copy)
                    if c % 4 == 3 or c == nchunk - 1:
                        g0 = (c // 4) * CCH
                        gw = min(CCH, CSLOT - g0)
                        mr = fp_.tile([1, CCH], BF16, tag="mr")
                        nc.sync.dma_start(mr[:, :gw], maskr[:, g0:g0 + gw])
                        psz = ps_c.tile([64, CCH], F32, tag="psc")
                        nc.tensor.matmul(psz[:, :gw], lhsT=mw1_t[:],
                                         rhs=z1t[:, :gw], start=True, stop=False)
                        nc.tensor.matmul(psz[:, :gw], lhsT=mb1r_t[:],
                                         rhs=onesc_t[:, :gw],
                                         start=False, stop=True)
                        zs2 = wp.tile([64, CCH], BF16, tag="zs2")
                        nc.vector.tensor_scalar(zs2[:, :gw], psz[:, :gw],
                                                SLOPE, None, ALU.mult)
                        z2 = wp.tile([64, CCH], BF16, tag="z2")
                        nc.vector.tensor_tensor(z2[:, :gw], psz[:, :gw],
                                                zs2[:, :gw], ALU.max)
                        py = ps_y.tile([1, CCH], F32, tag="psy")
                        nc.tensor.matmul(py[:, :gw], lhsT=mw2_t[:],
                                         rhs=z2[:, :gw], start=True, stop=False)
                        nc.tensor.matmul(py[:, :gw], lhsT=mb2r_t[:],
                                         rhs=onesc_t[:, :gw],
                                         start=False, stop=True)
                        ym = fp_.tile([1, CCH], F32, tag="ym")
                        nc.vector.tensor_tensor(ym[:, :gw], py[:, :gw],
                                                mr[:, :gw], ALU.add)
                        nc.sync.dma_start(ysh[:, g0:g0 + gw], ym[:, :gw])
            nc.gpsimd.collective_compute(
                "AllGather", ALU.bypass, replica_groups=rg,
                ins=[ysh[:].opt()], outs=[yfull[:].opt()])
            # ---- softmax over yfull viewed as [128, YCOLS] ----
            yf = bp.tile([P, YCOLS], F32, tag="yf")
            nc.sync.dma_start(yf[:], yfull[:].rearrange("a b -> (a b)")
                              .rearrange("(p c) -> p c", p=P))
            nc.sync.dma_start(y_out[:], yf[:])
            rmax = wp.tile([P, 1], F32, tag="rmax")
            nc.vector.tensor_reduce(rmax[:], yf[:], mybir.AxisListType.X, ALU.max)
            gmax = wp.tile([P, 1], F32, tag="gmax")
            nc.gpsimd.partition_all_reduce(gmax[:], rmax[:], P,
                                           bass_isa.ReduceOp.max)
            ngmax = wp.tile([P, 1], F32, tag="ngmax")
            nc.vector.tensor_scalar(ngmax[:], gmax[:], -1.0, None, ALU.mult)
            ef = bp.tile([P, YCOLS], F32, tag="ef")
            se = wp.tile([P, 1], F32, tag="se")
            nc.scalar.activation(ef[:], yf[:], AF.Exp, bias=ngmax[:, 0:1],
                                 accum_out=se[:])
            stot = wp.tile([P, 1], F32, tag="stot")
            nc.gpsimd.partition_all_reduce(stot[:], se[:], P,
                                           bass_isa.ReduceOp.add)
            invs = wp.tile([P, 1], F32, tag="invs")
            nc.vector.reciprocal(invs[:], stot[:])
            pf = bp.tile([P, YCOLS], F32, tag="pf")
            nc.vector.tensor_scalar(pf[:], ef[:], invs[:, 0:1], None, ALU.mult)
            nc.sync.dma_start(p_out[:], pf[:])
    nc.compile()
    return nc


def kernel(x, src, dst, cand_u, cand_v, cand_feat,
           w_self0, w_neigh0, b0, gamma0, beta0, rm0, rv0,
           w_self1, w_neigh1, b1, gamma1, beta1, rm1, rv1,
           mw0, mb0, mw1, mb1, mw2, mb2):
    x = np.asarray(x, np.float32)
    src = np.asarray(src, np.int64)
    dst = np.asarray(dst, np.int64)
    cand_u = np.asarray(cand_u, np.int64)
    cand_v = np.asarray(cand_v, np.int64)
    cand_feat = np.asarray(cand_feat, np.float32)

    deg = np.bincount(dst, minlength=N).astype(np.float32)
    invdeg = 1.0 / np.maximum(deg, 1.0)
    em, edata = _prep_edges(src, dst)
    cm, cdata = _prep_cands(cand_u, cand_v, cand_feat)

    xpad = np.zeros((NTOT, D), np.float32)
    xpad[:N] = x
    invdp = np.zeros(NTOT, np.float32)
    invdp[:N] = invdeg
    # permuted gather table: [part0: 8 x rows 0:3200 | part1: 8 x rows 3200:]
    gids = np.arange(NTOT)
    pp, tix = _node_part(gids)
    xperm = np.zeros((NTOT, D), np.float32)
    xperm[np.where(pp == 0, tix, T0 + tix)] = xpad[gids]

    com = {
        "xb": xperm.astype(BF),
        "identb": np.eye(P, dtype=np.float32).astype(BF),
        "onesr": np.ones((1, P), BF),
        "onesc": np.ones((1, CCH), BF),
    }
    for l, (ws, wn, b, ga, be, rme, rve) in enumerate(
        ((w_self0, w_neigh0, b0, gamma0, beta0, rm0, rv0),
         (w_self1, w_neigh1, b1, gamma1, beta1, rm1, rv1))):
        a = (np.asarray(ga) / np.sqrt(np.asarray(rve) + BN_EPS)).astype(np.float32)
        com[f"wself{l}"] = (np.asarray(ws) * a[None, :]).astype(BF)
        com[f"wneigh{l}"] = (np.asarray(wn) * a[None, :]).astype(BF)
        com[f"crow{l}"] = (a * (np.asarray(b) - np.asarray(rme)) + np.asarray(be)).astype(BF)[None, :]
    com["amat"] = np.asarray(mw0[0:128], np.float32).astype(BF)
    com["bmat"] = np.asarray(mw0[128:256], np.float32).astype(BF)
    com["mw0r"] = np.asarray(mw0[256], np.float32).astype(BF)[None, :]
    com["mb0r"] = np.asarray(mb0, np.float32).astype(BF)[None, :]
    com["mw1"] = np.asarray(mw1, np.float32).astype(BF)
    com["mb1r"] = np.asarray(mb1, np.float32).astype(BF)[None, :]
    com["mw2c"] = np.asarray(mw2, np.float32).astype(BF)
    com["mb2r"] = np.asarray(mb2, np.float32).reshape(1, 1).astype(BF)

    nc = _build_nc(em, cm)
    in_maps = []
    for k in range(NCORE):
        m = dict(com)
        m["xT"] = xpad[k * NSH:(k + 1) * NSH].T.astype(BF).copy()
        m["invd"] = np.tile(invdp[k * NSH:(k + 1) * NSH].astype(BF), (P, 1))
        m["gidx"] = edata[k]["gidx"]
        m["ohm"] = edata[k]["ohm"]
        m["cu"] = cdata[k]["cu"]
        m["cv"] = cdata[k]["cv"]
        m["featr"] = cdata[k]["feat"]
        m["maskr"] = cdata[k]["mask"]
        in_maps.append(m)
    trace = bool(os.environ.get("KERNEL_TRACE"))
    if trace:
        import types
        if "antenv.axon_hooks" not in sys.modules:
            try:
                import antenv
                from trn_agent_boot.trn_boot import _ntff_profile_via_ctypes
                mod = types.ModuleType("antenv.axon_hooks")
                hook = [_ntff_profile_via_ctypes("/opt/axon/libaxon_pjrt.so")]
                mod.set_axon_ntff_profile_hook = lambda h: hook.__setitem__(0, h)
                mod.get_axon_ntff_profile_hook = lambda: hook[0]
                sys.modules["antenv.axon_hooks"] = mod
                antenv.axon_hooks = mod
            except Exception:
                trace = False
    res = run_bass_kernel_spmd(nc, in_maps, core_ids=list(range(NCORE)),
                               trace=trace,
                               tmpdir=os.environ.get("KERNEL_TRACE_DIR"))
    if trace and res.exec_time_ns is not None:
        print(f"HW exec time: {res.exec_time_ns} ns")
    y_all = np.zeros(C, np.float32)
    p_all = np.zeros(C, np.float32)
    cslot = cm["cslot"]
    y_lin = res.results[0]["y_out"].ravel()   # flat order: core, slot
    p_lin = res.results[0]["p_out"].ravel()
    for k in range(NCORE):
        sm = cdata[k]["slotmap"]
        valid = sm >= 0
        j = np.nonzero(valid)[0]
        y_all[sm[valid]] = y_lin[k * cslot + j]
        p_all[sm[valid]] = p_lin[k * cslot + j]
    return y_all[:, None], p_all[:, None]


# revision 16
# speedup vs baseline: 1.0556x; 1.0556x over previous
"""Trainium2 Bass kernel for nn_PolicyNetwork3 (2-layer GraphSAGE + edge-MLP).

v2 design (8 NeuronCores, SPMD single NEFF):
- dst-sharded aggregation; core k owns node block [6272k, 6272k+6272).
- Edge messages gathered per-edge from bf16 HBM row tables via 4-queue
  round-robin dma_gather (descriptor generation parallelizes across the
  SWDGE queues; ~2.4ns/row vs 8ns serialized).
- segment-sum per 256-dst window via one-hot matmuls (bf16 gathered rows x
  fp8 0/1 one-hot streamed from HBM) accumulating in PSUM across the LO/HI
  half-table runs; per-window PSUM->SBUF copy fuses the 1/deg scale.
- BN folded into SAGE weights on host; linear+leaky per 128-node block;
  h shards exchanged with bf16 AllGather.
- candidate MLP: g/q tables per node; transposed candidate gathers put
  features on partitions so the whole MLP runs as 512-wide PE matmuls
  (identity-matmul adds, K=1 bias rows); global softmax on-device.
"""

import os
import sys

sys.path.insert(0, "/opt/trn_rl_repo")
sys.path.insert(0, "/root/.axon_site")

import numpy as np
import ml_dtypes

import concourse.bacc as bacc
import concourse.bass as bass
import concourse.bass_isa as bass_isa
import concourse.mybir as mybir
import concourse.tile as tile
from concourse import library_config
from concourse.bass_utils import run_bass_kernel_spmd

P = 128
N, E, C = 50000, 800000, 100000
D = 128
NCORE = 8
NSH = 6272            # nodes per core shard
NTOT = NSH * NCORE    # 50176 padded node table
HALF = NTOT // 2      # 25088 rows per gather-table half
PART0 = 3200          # local rows in table part 0 (25 blocks)
PART1 = NSH - PART0   # 3072 local rows in part 1
T0 = NCORE * PART0    # 25600 rows in part-0 table
T1 = NCORE * PART1    # 24576 rows in part-1 table
W = 256               # dst window width (PSUM cols)
NWIN = (NSH + W - 1) // W   # 25 windows (last one 128 wide)
NBLK = NSH // P       # 49 node blocks for the linear phase
CSH = C // NCORE      # 12500 candidates per core
GCALL = 2048          # max idxs per dma_gather call
CCH = 512             # candidate MLP chunk
BN_EPS = 1e-5
SLOPE = 0.01
F32 = mybir.dt.float32
BF16 = mybir.dt.bfloat16
F8 = mybir.dt.float8e4
I16 = mybir.dt.int16
AF = mybir.ActivationFunctionType
ALU = mybir.AluOpType
BF = ml_dtypes.bfloat16
F8NP = ml_dtypes.float8_e4m3fn


def _wrap16(idx_lin):
    """[n] -> [128, n/16] int16 in the dma_gather wrapped+replicated layout."""
    n = idx_lin.shape[0]
    assert n % 16 == 0
    w = idx_lin.reshape(n // 16, 16).T.astype(np.int16)
    return np.tile(w, (8, 1)).copy()


def _win_width(w):
    return min(W, NSH - w * W)


def _node_part(g):
    """Global node id -> (part, int16 table index) under the shard-half
    permuted table layout [part0: 8 x rows 0:3200 | part1: 8 x rows 3200:]."""
    k, r = g // NSH, g % NSH
    p = (r >= PART0).astype(np.int64)
    idx = np.where(p == 0, k * PART0 + r, k * PART1 + (r - PART0))
    return p, idx


def _prep_edges(src, dst):
    """Uniform per-core window/run/chunk schedule + per-core idx and one-hot."""
    core = dst // NSH
    winl = (dst - core * NSH) // W
    dstloc = (dst - core * NSH) - winl * W
    half, tidx = _node_part(src)
    key = (core * NWIN + winl) * 2 + half
    order = np.argsort(key, kind="stable")
    cnt = np.bincount(key, minlength=NCORE * NWIN * 2).reshape(NCORE, NWIN, 2)
    nch_u = (-(-cnt // P)).max(axis=0)        # [NWIN, 2] uniform chunk counts
    # global chunk order: w0 LO chunks, w0 HI, w1 LO, ...
    win_ch0 = np.zeros(NWIN + 1, np.int64)
    np.cumsum(nch_u.sum(axis=1), out=win_ch0[1:])
    totch = int(win_ch0[-1])
    nslot = totch * P
    # gather calls (uniform): per (w, half) run split into balanced pieces
    # of ~CTGT chunks so the 4 SWDGE queues stay evenly loaded
    CTGT = 8
    calls = {}  # (w, s) -> list of (slot_start, n_idx, chunk0)
    for w in range(NWIN):
        c0 = int(win_ch0[w])
        for s in (0, 1):
            nch = int(nch_u[w, s])
            cl = []
            if nch > 0:
                npiece = -(-nch // CTGT)
                base, rem = divmod(nch, npiece)
                cc = c0
                for i in range(npiece):
                    sz = base + (1 if i < rem else 0)
                    cl.append((cc * P, sz * P, cc))
                    cc += sz
            calls[(w, s)] = cl
            c0 += nch
    bstart = np.zeros(NCORE * NWIN * 2 + 1, np.int64)
    np.cumsum(np.bincount(key, minlength=NCORE * NWIN * 2), out=bstart[1:])
    gidx = np.zeros((NCORE, nslot), np.int16)
    ohm = np.zeros((NCORE, P, totch * W), np.uint8)  # fp8 bits (1.0 = 0x38)
    ONE = np.float32(1.0).astype(F8NP).view(np.uint8)
    for k in range(NCORE):
        for w in range(NWIN):
            pos = int(win_ch0[w]) * P
            for s in (0, 1):
                b = (k * NWIN + w) * 2 + s
                sl = order[bstart[b]:bstart[b + 1]]
                n = len(sl)
                gidx[k, pos:pos + n] = tidx[sl].astype(np.int16)
                slots = pos + np.arange(n)
                ohm[k, slots % P, (slots // P) * W + dstloc[sl]] = ONE
                pos += int(nch_u[w, s]) * P
    meta = dict(nch_u=nch_u, win_ch0=win_ch0, totch=totch, nslot=nslot,
                calls=calls)
    data = [dict(gidx=_wrap16(gidx[k]), ohm=ohm[k].view(F8NP)) for k in range(NCORE)]
    return meta, data


def _prep_cands(cand_u, cand_v, cand_feat):
    """Shard candidates, group by (u_half, v_half), pad groups to 128."""
    gch = np.zeros((NCORE, 4), np.int64)
    groups = [[None] * 4 for _ in range(NCORE)]
    for k in range(NCORE):
        ids = np.arange(k * CSH, (k + 1) * CSH)
        pu, _ = _node_part(cand_u[ids])
        pv, _ = _node_part(cand_v[ids])
        g = pu * 2 + pv
        for gi in range(4):
            groups[k][gi] = ids[g == gi]
            gch[k, gi] = -(-len(groups[k][gi]) // P)
    gch_u = gch.max(axis=0)
    goff = np.zeros(5, np.int64)
    np.cumsum(gch_u * P, out=goff[1:])
    cslot = int(goff[4])                       # 128-granular
    cu = np.zeros((NCORE, cslot), np.int16)
    cv = np.zeros((NCORE, cslot), np.int16)
    ft = np.zeros((NCORE, cslot), BF)
    mask = np.full((NCORE, cslot), -1e30, np.float32).astype(BF)
    slotmap = np.full((NCORE, cslot), -1, np.int64)
    for k in range(NCORE):
        for gi in range(4):
            ids = groups[k][gi]
            n = len(ids)
            p0 = int(goff[gi])
            _, uix = _node_part(cand_u[ids])
            _, vix = _node_part(cand_v[ids])
            cu[k, p0:p0 + n] = uix.astype(np.int16)
            cv[k, p0:p0 + n] = vix.astype(np.int16)
            ft[k, p0:p0 + n] = cand_feat[ids, 0].astype(BF)
            mask[k, p0:p0 + n] = 0.0
            slotmap[k, p0:p0 + n] = ids
    # u calls: groups 0-1 (uh=0) then 2-3 (uh=1); v calls per group;
    # balanced ~8-chunk pieces
    CTGT = 8

    def _split(lo, hi, s, out):
        nch = (hi - lo) // P
        if nch <= 0:
            return
        npiece = -(-nch // CTGT)
        base, rem = divmod(nch, npiece)
        p = lo
        for i in range(npiece):
            sz = (base + (1 if i < rem else 0)) * P
            out.append((p, sz, s))
            p += sz

    ucalls, vcalls = [], []
    _split(0, int(goff[2]), 0, ucalls)
    _split(int(goff[2]), int(goff[4]), 1, ucalls)
    for gi in range(4):
        _split(int(goff[gi]), int(goff[gi + 1]), gi % 2, vcalls)
    meta = dict(cslot=cslot, ucalls=ucalls, vcalls=vcalls)
    data = [dict(cu=_wrap16(cu[k]), cv=_wrap16(cv[k]), feat=ft[k][None, :],
                 mask=mask[k][None, :], slotmap=slotmap[k]) for k in range(NCORE)]
    return meta, data


def _build_nc(em, cm):
    nc = bacc.Bacc("TRN2", target_bir_lowering=False, debug=False,
                   num_devices=NCORE, num_swdge_queues=4)
    TOTCH, NSLOT = em["totch"], em["nslot"]
    CSLOT = cm["cslot"]
    NCC = -(-CSLOT // CCH)                  # candidate MLP chunk groups
    YCOLS = NCORE * CSLOT // P              # yfull viewed as [128, YCOLS]

    xb = nc.dram_tensor("xb", [NTOT, D], BF16, kind="ExternalInput")
    xT = nc.dram_tensor("xT", [P, NSH], BF16, kind="ExternalInput")
    gidx = nc.dram_tensor("gidx", [P, NSLOT // 16], I16, kind="ExternalInput")
    ohm = nc.dram_tensor("ohm", [P, TOTCH * W], F8, kind="ExternalInput")
    invd = nc.dram_tensor("invd", [P, NSH], BF16, kind="ExternalInput")
    wself = [nc.dram_tensor(f"wself{l}", [D, D], BF16, kind="ExternalInput") for l in range(2)]
    wneigh = [nc.dram_tensor(f"wneigh{l}", [D, D], BF16, kind="ExternalInput") for l in range(2)]
    crow = [nc.dram_tensor(f"crow{l}", [1, D], BF16, kind="ExternalInput") for l in range(2)]
    identb = nc.dram_tensor("identb", [P, P], BF16, kind="ExternalInput")
    onesr = nc.dram_tensor("onesr", [1, P], BF16, kind="ExternalInput")
    onesc = nc.dram_tensor("onesc", [1, CCH], BF16, kind="ExternalInput")
    amat = nc.dram_tensor("amat", [D, 64], BF16, kind="ExternalInput")
    bmat = nc.dram_tensor("bmat", [D, 64], BF16, kind="ExternalInput")
    mw0r = nc.dram_tensor("mw0r", [1, 64], BF16, kind="ExternalInput")
    mb0r = nc.dram_tensor("mb0r", [1, 64], BF16, kind="ExternalInput")
    mw1 = nc.dram_tensor("mw1", [64, 64], BF16, kind="ExternalInput")
    mb1r = nc.dram_tensor("mb1r", [1, 64], BF16, kind="ExternalInput")
    mw2 = nc.dram_tensor("mw2c", [64, 1], BF16, kind="ExternalInput")
    mb2r = nc.dram_tensor("mb2r", [1, 1], BF16, kind="ExternalInput")
    cu = nc.dram_tensor("cu", [P, CSLOT // 16], I16, kind="ExternalInput")
    cv = nc.dram_tensor("cv", [P, CSLOT // 16], I16, kind="ExternalInput")
    featr = nc.dram_tensor("featr", [1, CSLOT], BF16, kind="ExternalInput")
    maskr = nc.dram_tensor("maskr", [1, CSLOT], BF16, kind="ExternalInput")

    y_out = nc.dram_tensor("y_out", [P, YCOLS], F32, kind="ExternalOutput")
    p_out = nc.dram_tensor("p_out", [P, YCOLS], F32, kind="ExternalOutput")

    hshA = nc.dram_tensor("hshA", [PART0, D], BF16, kind="Internal")
    hshB = nc.dram_tensor("hshB", [PART1, D], BF16, kind="Internal")
    hfull0 = nc.dram_tensor("hfull0", [T0, D], BF16, kind="Internal",
                            addr_space="Shared")
    hfull1 = nc.dram_tensor("hfull1", [T1, D], BF16, kind="Internal",
                            addr_space="Shared")
    gqshA = nc.dram_tensor("gqshA", [PART0, D], BF16, kind="Internal")
    gqshB = nc.dram_tensor("gqshB", [PART1, D], BF16, kind="Internal")
    gqfull0 = nc.dram_tensor("gqfull0", [T0, D], BF16, kind="Internal",
                             addr_space="Shared")
    gqfull1 = nc.dram_tensor("gqfull1", [T1, D], BF16, kind="Internal",
                             addr_space="Shared")
    ysh = nc.dram_tensor("ysh", [1, CSLOT], F32, kind="Internal")
    yfull = nc.dram_tensor("yfull", [NCORE, CSLOT], F32, kind="Internal",
                           addr_space="Shared")

    rg = [list(range(NCORE))]
    nch_u, win_ch0, calls = em["nch_u"], em["win_ch0"], em["calls"]

    with tile.TileContext(nc) as tc:
        with (
            tc.tile_pool(name="const", bufs=1) as cp,
            tc.tile_pool(name="big", bufs=1) as bp,
            tc.tile_pool(name="msgs", bufs=9) as mp,
            tc.tile_pool(name="ohp", bufs=4) as op_,
            tc.tile_pool(name="wrk", bufs=4) as wp,
            tc.tile_pool(name="frows", bufs=3) as fp_,
            tc.tile_pool(name="gup", bufs=3) as gup,
            tc.tile_pool(name="gvp", bufs=3) as gvp,
            tc.tile_pool(name="z1p", bufs=2) as z1p,
        ):
            nc.gpsimd.load_library(library_config.mlp)

            def load(pool, t, shape=None):
                tl = pool.tile(shape or list(t.shape), t.dtype, tag=t.name)
                nc.sync.dma_start(tl[:], t[:])
                return tl

            gidx_t = load(cp, gidx)
            invd_t = load(cp, invd)
            identb_t = load(cp, identb)
            onesr_t = load(cp, onesr)
            onesc_t = load(cp, onesc)
            wself_t = [load(cp, t) for t in wself]
            wneigh_t = [load(cp, t) for t in wneigh]
            crow_t = [load(cp, t) for t in crow]
            amat_t = load(cp, amat)
            bmat_t = load(cp, bmat)
            mw0r_t = load(cp, mw0r)
            mb0r_t = load(cp, mb0r)
            mw1_t = load(cp, mw1)
            mb1r_t = load(cp, mb1r)
            mw2_t = load(cp, mw2)
            mb2r_t = load(cp, mb2r)
            cu_t = load(cp, cu)
            cv_t = load(cp, cv)
            featr_t = load(cp, featr)

            curT = bp.tile([P, NSH], BF16, tag="curT")
            nxtT = bp.tile([P, NSH], BF16, tag="nxtT")
            aggrb = bp.tile([P, NSH], BF16, tag="aggrb")
            nc.sync.dma_start(curT[:], xT[:])

            qrr = [0]

            def next_q():
                q = qrr[0]
                qrr[0] = (q + 1) % 4
                return q

            ctxA = tc.tile_pool(name="ps_w", bufs=3, space="PSUM")
            ps_w = ctxA.__enter__()
            ctxB = tc.tile_pool(name="ps_l", bufs=2, space="PSUM")
            ps_l = ctxB.__enter__()
            ctxC = tc.tile_pool(name="ps_t", bufs=1, space="PSUM")
            ps_t = ctxC.__enter__()

            def linear_block(layer, b):
                ph = ps_l.tile([P, P], F32, tag="ph")
                nc.tensor.matmul(ph[:], lhsT=aggrb[:, b * P:(b + 1) * P],
                                 rhs=wneigh_t[layer][:], start=True, stop=False)
                nc.tensor.matmul(ph[:], lhsT=curT[0][:, b * P:(b + 1) * P],
                                 rhs=wself_t[layer][:], start=False, stop=False)
                nc.tensor.matmul(ph[:], lhsT=onesr_t[:], rhs=crow_t[layer][:],
                                 start=False, stop=True)
                tmp = wp.tile([P, P], BF16, tag="tmp")
                nc.vector.tensor_scalar(tmp[:], ph[:], SLOPE, None, ALU.mult)
                ht = wp.tile([P, P], BF16, tag="ht")
                nc.vector.tensor_tensor(ht[:], ph[:], tmp[:], ALU.max)
                if layer == 0:
                    if b < PART0 // P:
                        nc.sync.dma_start(hshA[b * P:(b + 1) * P, :], ht[:])
                    else:
                        b1 = b - PART0 // P
                        nc.sync.dma_start(hshB[b1 * P:(b1 + 1) * P, :], ht[:])
                pt = ps_t.tile([P, P], BF16, tag="pt")
                nc.tensor.transpose(pt[:], ht[:], identb_t[:])
                nc.scalar.activation(nxtT[0][:, b * P:(b + 1) * P], pt[:],
                                     AF.Copy)
                if layer == 0:
                    if b == PART0 // P - 1:
                        nc.gpsimd.collective_compute(
                            "AllGather", ALU.bypass, replica_groups=rg,
                            ins=[hshA[:].opt()], outs=[hfull0[:].opt()])
                else:
                    pg = ps_l.tile([P, P], F32, tag="ph")
                    nc.tensor.matmul(pg[:, 0:64],
                                     lhsT=nxtT[0][:, b * P:(b + 1) * P],
                                     rhs=amat_t[:], start=True, stop=True)
                    nc.tensor.matmul(pg[:, 64:128],
                                     lhsT=nxtT[0][:, b * P:(b + 1) * P],
                                     rhs=bmat_t[:], start=True, stop=False)
                    nc.tensor.matmul(pg[:, 64:128], lhsT=onesr_t[:],
                                     rhs=mb0r_t[:], start=False, stop=True)
                    gq = wp.tile([P, P], BF16, tag="gq")
                    nc.scalar.activation(gq[:], pg[:], AF.Copy)
                    if b < PART0 // P:
                        nc.sync.dma_start(gqshA[b * P:(b + 1) * P, :], gq[:])
                    else:
                        b1 = b - PART0 // P
                        nc.sync.dma_start(gqshB[b1 * P:(b1 + 1) * P, :], gq[:])
                    if b == PART0 // P - 1:
                        nc.gpsimd.collective_compute(
                            "AllGather", ALU.bypass, replica_groups=rg,
                            ins=[gqshA[:].opt()],
                            outs=[gqfull0[:].opt()])

            def run_gathers(ps, gtab_s, cl, first, mtot):
                done = first
                for (p0, n_idx, ch0) in cl:
                    ncall = n_idx // P
                    g = mp.tile([P, GCALL // P, P], BF16, tag="g")
                    nc.gpsimd.dma_gather(
                        g[:, :ncall, :], gtab_s[:],
                        gidx_t[:, p0 // 16:(p0 + n_idx) // 16],
                        n_idx, n_idx, P, single_packet=False,
                        queue_num=next_q())
                    oht = op_.tile([P, GCALL // P, W], F8, tag="oh")
                    nc.sync.dma_start(
                        oht[:, :ncall, :],
                        ohm[:, ch0 * W:(ch0 + ncall) * W]
                        .rearrange("p (a b) -> p a b", b=W))
                    for cc in range(ncall):
                        nc.tensor.matmul(
                            ps[:], lhsT=g[:, cc, :], rhs=oht[:, cc, :],
                            start=(done == 0), stop=(done == mtot - 1))
                        done += 1

            curT, nxtT = [curT], [nxtT]
            for layer in range(2):
                tabs = ((xb[0:T0, :], xb[T0:NTOT, :]) if layer == 0
                        else (hfull0[:], hfull1[:]))
                if layer == 0:
                    # single pass: both parts accumulate into one PSUM group
                    for w in range(NWIN):
                        ww = _win_width(w)
                        wsl = aggrb[:, w * W:w * W + ww]
                        nch0 = int(nch_u[w, 0])
                        mtot = nch0 + int(nch_u[w, 1])
                        if mtot == 0:
                            nc.vector.memset(wsl, 0.0)
                        else:
                            ps = ps_w.tile([P, W], F32, tag="psw")
                            run_gathers(ps, tabs[0], calls[(w, 0)], 0, mtot)
                            run_gathers(ps, tabs[1], calls[(w, 1)], nch0, mtot)
                            nc.vector.tensor_tensor(
                                wsl, ps[:, :ww],
                                invd_t[:, w * W:w * W + ww], ALU.mult)
                        for b in range((w * W) // P, (w * W + ww) // P):
                            linear_block(layer, b)
                else:
                    # pass 0: part-0 runs (overlap AG of hfull1)
                    for w in range(NWIN):
                        ww = _win_width(w)
                        nch = int(nch_u[w, 0])
                        if nch == 0:
                            nc.vector.memset(aggrb[:, w * W:w * W + ww], 0.0)
                            continue
                        ps = ps_w.tile([P, W], F32, tag="psw")
                        run_gathers(ps, tabs[0], calls[(w, 0)], 0, nch)
                        nc.scalar.activation(aggrb[:, w * W:w * W + ww],
                                             ps[:, :ww], AF.Copy)
                    # pass 1: part-1 runs -> add + 1/deg scale -> linear
                    for w in range(NWIN):
                        ww = _win_width(w)
                        wsl = aggrb[:, w * W:w * W + ww]
                        nch = int(nch_u[w, 1])
                        if nch > 0:
                            ps = ps_w.tile([P, W], F32, tag="psw")
                            run_gathers(ps, tabs[1], calls[(w, 1)], 0, nch)
                            nc.vector.tensor_tensor(wsl, wsl, ps[:, :ww],
                                                    ALU.add)
                        nc.vector.tensor_tensor(wsl, wsl,
                                                invd_t[:, w * W:w * W + ww],
                                                ALU.mult)
                        for b in range((w * W) // P, (w * W + ww) // P):
                            linear_block(layer, b)
                if layer == 0:
                    nc.gpsimd.collective_compute(
                        "AllGather", ALU.bypass, replica_groups=rg,
                        ins=[hshB[:].opt()], outs=[hfull1[:].opt()])
                curT[0], nxtT[0] = nxtT[0], curT[0]
            nc.gpsimd.collective_compute(
                "AllGather", ALU.bypass, replica_groups=rg,
                ins=[gqshB[:].opt()], outs=[gqfull1[:].opt()])

            ctxC.__exit__(None, None, None)
            ctxB.__exit__(None, None, None)
            ctxA.__exit__(None, None, None)
            with (
                tc.tile_pool(name="psc1", bufs=2, space="PSUM") as psc1,
                tc.tile_pool(name="psc2", bufs=2, space="PSUM") as psc2,
                tc.tile_pool(name="ps_c", bufs=2, space="PSUM") as ps_c,
                tc.tile_pool(name="ps_y", bufs=1, space="PSUM") as ps_y,
            ):
                # ---- candidate gathers (slot layout, 4-queue) ----
                ucalls, vcalls = cm["ucalls"], cm["vcalls"]
                uci, vci = [], []
                for i, (s0, n, sh) in enumerate(ucalls):
                    uci += [(i, j) for j in range(n // P)]
                for i, (s0, n, sh) in enumerate(vcalls):
                    vci += [(i, j) for j in range(n // P)]
                gu_tiles = [None] * len(ucalls)
                gv_tiles = [None] * len(vcalls)
                for i in range(max(len(ucalls), len(vcalls))):
                    for (tiles, calls, pool, idx_t) in (
                            (gu_tiles, ucalls, gup, cu_t),
                            (gv_tiles, vcalls, gvp, cv_t)):
                        if i >= len(calls):
                            continue
                        (s0, n_idx, sh) = calls[i]
                        ncall = n_idx // P
                        g = pool.tile([P, GCALL // P, P], BF16, tag="g")
                        nc.gpsimd.dma_gather(
                            g[:, :ncall, :],
                            (gqfull0 if sh == 0 else gqfull1)[:],
                            idx_t[:, s0 // 16:(s0 + n_idx) // 16],
                            n_idx, n_idx, P, single_packet=False,
                            queue_num=next_q())
                        tiles[i] = g
                # ---- candidate MLP ----
                nchunk = CSLOT // P
                z1t = None
                for c in range(nchunk):
                    ui, uo = uci[c]
                    vi, vo = vci[c]
                    psf = psc1.tile([P, 64], F32, tag="psf")
                    nc.tensor.matmul(psf[:], lhsT=identb_t[:],
                                     rhs=gu_tiles[ui][:, uo, 0:64],
                                     start=True, stop=False)
                    nc.tensor.matmul(psf[:], lhsT=identb_t[:],
                                     rhs=gv_tiles[vi][:, vo, 64:128],
                                     start=False, stop=False)
                    nc.tensor.matmul(psf[:], lhsT=featr_t[:, c * P:(c + 1) * P],
                                     rhs=mw0r_t[:], start=False, stop=True)
                    zs = wp.tile([P, 64], BF16, tag="zs")
                    nc.vector.tensor_scalar(zs[:], psf[:], SLOPE, None, ALU.mult)
                    z1b = wp.tile([P, 64], BF16, tag="z1b")
                    nc.vector.tensor_tensor(z1b[:], psf[:], zs[:], ALU.max)
                    ptz = psc2.tile([64, P], BF16, tag="ptz")
                    nc.tensor.transpose(ptz[:], z1b[:], identb_t[:])
                    if c % 4 == 0:
                        z1t = z1p.tile([64, CCH], BF16, tag="z1t")
                    nc.scalar.activation(z1t[:, (c % 4) * P:(c % 4 + 1) * P],
                                         ptz[:], AF.Copy)
                    if c % 4 == 3 or c == nchunk - 1:
                        g0 = (c // 4) * CCH
                        gw = min(CCH, CSLOT - g0)
                        mr = fp_.tile([1, CCH], BF16, tag="mr")
                        nc.sync.dma_start(mr[:, :gw], maskr[:, g0:g0 + gw])
                        psz = ps_c.tile([64, CCH], F32, tag="psc")
                        nc.tensor.matmul(psz[:, :gw], lhsT=mw1_t[:],
                                         rhs=z1t[:, :gw], start=True, stop=False)
                        nc.tensor.matmul(psz[:, :gw], lhsT=mb1r_t[:],
                                         rhs=onesc_t[:, :gw],
                                         start=False, stop=True)
                        zs2 = wp.tile([64, CCH], BF16, tag="zs2")
                        nc.vector.tensor_scalar(zs2[:, :gw], psz[:, :gw],
                                                SLOPE, None, ALU.mult)
                        z2 = wp.tile([64, CCH], BF16, tag="z2")
                        nc.vector.tensor_tensor(z2[:, :gw], psz[:, :gw],
                                                zs2[:, :gw], ALU.max)
                        py = ps_y.tile([1, CCH], F32, tag="psy")
                        nc.tensor.matmul(py[:, :gw], lhsT=mw2_t[:],
                                         rhs=z2[:, :gw], start=True, stop=False)
                        nc.tensor.matmul(py[:, :gw], lhsT=mb2r_t[:],
                                         rhs=onesc_t[:, :gw],
                                         start=False, stop=True)
                        ym = fp_.tile([1, CCH], F32, tag="ym")
                        nc.vector.tensor_tensor(ym[:, :gw], py[:, :gw],
                                                mr[:, :gw], ALU.add)
                        nc.sync.dma_start(ysh[:, g0:g0 + gw], ym[:, :gw])
            nc.gpsimd.collective_compute(
                "AllGather", ALU.bypass, replica_groups=rg,
                ins=[ysh[:].opt()], outs=[yfull[:].opt()])
            # ---- softmax over yfull viewed as [128, YCOLS] ----
            yf = bp.tile([P, YCOLS], F32, tag="yf")
            nc.sync.dma_start(yf[:], yfull[:].rearrange("a b -> (a b)")
                              .rearrange("(p c) -> p c", p=P))
            nc.sync.dma_start(y_out[:], yf[:])
            rmax = wp.tile([P, 1], F32, tag="rmax")
            nc.vector.tensor_reduce(rmax[:], yf[:], mybir.AxisListType.X, ALU.max)
            gmax = wp.tile([P, 1], F32, tag="gmax")
            nc.gpsimd.partition_all_reduce(gmax[:], rmax[:], P,
                                           bass_isa.ReduceOp.max)
            ngmax = wp.tile([P, 1], F32, tag="ngmax")
            nc.vector.tensor_scalar(ngmax[:], gmax[:], -1.0, None, ALU.mult)
            ef = bp.tile([P, YCOLS], F32, tag="ef")
            se = wp.tile([P, 1], F32, tag="se")
            nc.scalar.activation(ef[:], yf[:], AF.Exp, bias=ngmax[:, 0:1],
                                 accum_out=se[:])
            stot = wp.tile([P, 1], F32, tag="stot")
            nc.gpsimd.partition_all_reduce(stot[:], se[:], P,
                                           bass_isa.ReduceOp.add)
            invs = wp.tile([P, 1], F32, tag="invs")
            nc.vector.reciprocal(invs[:], stot[:])
            pf = bp.tile([P, YCOLS], F32, tag="pf")
            nc.vector.tensor_scalar(pf[:], ef[:], invs[:, 0:1], None, ALU.mult)
            nc.sync.dma_start(p_out[:], pf[:])
    nc.compile()
    return nc


def kernel(x, src, dst, cand_u, cand_v, cand_feat,
           w_self0, w_neigh0, b0, gamma0, beta0, rm0, rv0,
           w_self1, w_neigh1, b1, gamma1, beta1, rm1, rv1,
           mw0, mb0, mw1, mb1, mw2, mb2):
    x = np.asarray(x, np.float32)
    src = np.asarray(src, np.int64)
    dst = np.asarray(dst, np.int64)
    cand_u = np.asarray(cand_u, np.int64)
    cand_v = np.asarray(cand_v, np.int64)
    cand_feat = np.asarray(cand_feat, np.float32)

    deg = np.bincount(dst, minlength=N).astype(np.float32)
    invdeg = 1.0 / np.maximum(deg, 1.0)
    em, edata = _prep_edges(src, dst)
    cm, cdata = _prep_cands(cand_u, cand_v, cand_feat)

    xpad = np.zeros((NTOT, D), np.float32)
    xpad[:N] = x
    invdp = np.zeros(NTOT, np.float32)
    invdp[:N] = invdeg
    # permuted gather table: [part0: 8 x rows 0:3200 | part1: 8 x rows 3200:]
    gids = np.arange(NTOT)
    pp, tix = _node_part(gids)
    xperm = np.zeros((NTOT, D), np.float32)
    xperm[np.where(pp == 0, tix, T0 + tix)] = xpad[gids]

    com = {
        "xb": xperm.astype(BF),
        "identb": np.eye(P, dtype=np.float32).astype(BF),
        "onesr": np.ones((1, P), BF),
        "onesc": np.ones((1, CCH), BF),
    }
    for l, (ws, wn, b, ga, be, rme, rve) in enumerate(
        ((w_self0, w_neigh0, b0, gamma0, beta0, rm0, rv0),
         (w_self1, w_neigh1, b1, gamma1, beta1, rm1, rv1))):
        a = (np.asarray(ga) / np.sqrt(np.asarray(rve) + BN_EPS)).astype(np.float32)
        com[f"wself{l}"] = (np.asarray(ws) * a[None, :]).astype(BF)
        com[f"wneigh{l}"] = (np.asarray(wn) * a[None, :]).astype(BF)
        com[f"crow{l}"] = (a * (np.asarray(b) - np.asarray(rme)) + np.asarray(be)).astype(BF)[None, :]
    com["amat"] = np.asarray(mw0[0:128], np.float32).astype(BF)
    com["bmat"] = np.asarray(mw0[128:256], np.float32).astype(BF)
    com["mw0r"] = np.asarray(mw0[256], np.float32).astype(BF)[None, :]
    com["mb0r"] = np.asarray(mb0, np.float32).astype(BF)[None, :]
    com["mw1"] = np.asarray(mw1, np.float32).astype(BF)
    com["mb1r"] = np.asarray(mb1, np.float32).astype(BF)[None, :]
    com["mw2c"] = np.asarray(mw2, np.float32).astype(BF)
    com["mb2r"] = np.asarray(mb2, np.float32).reshape(1, 1).astype(BF)

    nc = _build_nc(em, cm)
    in_maps = []
    for k in range(NCORE):
        m = dict(com)
        m["xT"] = xpad[k * NSH:(k + 1) * NSH].T.astype(BF).copy()
        m["invd"] = np.tile(invdp[k * NSH:(k + 1) * NSH].astype(BF), (P, 1))
        m["gidx"] = edata[k]["gidx"]
        m["ohm"] = edata[k]["ohm"]
        m["cu"] = cdata[k]["cu"]
        m["cv"] = cdata[k]["cv"]
        m["featr"] = cdata[k]["feat"]
        m["maskr"] = cdata[k]["mask"]
        in_maps.append(m)
    trace = bool(os.environ.get("KERNEL_TRACE"))
    if trace:
        import types
        if "antenv.axon_hooks" not in sys.modules:
            try:
                import antenv
                from trn_agent_boot.trn_boot import _ntff_profile_via_ctypes
                mod = types.ModuleType("antenv.axon_hooks")
                hook = [_ntff_profile_via_ctypes("/opt/axon/libaxon_pjrt.so")]
                mod.set_axon_ntff_profile_hook = lambda h: hook.__setitem__(0, h)
                mod.get_axon_ntff_profile_hook = lambda: hook[0]
                sys.modules["antenv.axon_hooks"] = mod
                antenv.axon_hooks = mod
            except Exception:
                trace = False
    res = run_bass_kernel_spmd(nc, in_maps, core_ids=list(range(NCORE)),
                               trace=trace,
                               tmpdir=os.environ.get("KERNEL_TRACE_DIR"))
    if trace and res.exec_time_ns is not None:
        print(f"HW exec time: {res.exec_time_ns} ns")
    y_all = np.zeros(C, np.float32)
    p_all = np.zeros(C, np.float32)
    cslot = cm["cslot"]
    y_lin = res.results[0]["y_out"].ravel()   # flat order: core, slot
    p_lin = res.results[0]["p_out"].ravel()
    for k in range(NCORE):
        sm = cdata[k]["slotmap"]
        valid = sm >= 0
        j = np.nonzero(valid)[0]
        y_all[sm[valid]] = y_lin[k * cslot + j]
        p_all[sm[valid]] = p_lin[k * cslot + j]
    return y_all[:, None], p_all[:, None]


# revision 17
# speedup vs baseline: 1.1208x; 1.0617x over previous
"""Trainium2 Bass kernel for nn_PolicyNetwork3 (2-layer GraphSAGE + edge-MLP).

v2 design (8 NeuronCores, SPMD single NEFF):
- dst-sharded aggregation; core k owns node block [6272k, 6272k+6272).
- Edge messages gathered per-edge from bf16 HBM row tables via 4-queue
  round-robin dma_gather (descriptor generation parallelizes across the
  SWDGE queues; ~2.4ns/row vs 8ns serialized).
- segment-sum per 256-dst window via one-hot matmuls (bf16 gathered rows x
  fp8 0/1 one-hot streamed from HBM) accumulating in PSUM across the LO/HI
  half-table runs; per-window PSUM->SBUF copy fuses the 1/deg scale.
- BN folded into SAGE weights on host; linear+leaky per 128-node block;
  h shards exchanged with bf16 AllGather.
- candidate MLP: g/q tables per node; transposed candidate gathers put
  features on partitions so the whole MLP runs as 512-wide PE matmuls
  (identity-matmul adds, K=1 bias rows); global softmax on-device.
"""

import os
import sys

sys.path.insert(0, "/opt/trn_rl_repo")
sys.path.insert(0, "/root/.axon_site")

import numpy as np
import ml_dtypes

import concourse.bacc as bacc
import concourse.bass as bass
import concourse.bass_isa as bass_isa
import concourse.mybir as mybir
import concourse.tile as tile
from concourse import library_config
from concourse.bass_utils import run_bass_kernel_spmd

P = 128
N, E, C = 50000, 800000, 100000
D = 128
NCORE = 8
NSH = 6272            # nodes per core shard
NTOT = NSH * NCORE    # 50176 padded node table
HALF = NTOT // 2      # 25088 rows per gather-table half
PART0 = 3200          # local rows in table part 0 (25 blocks)
PART1 = NSH - PART0   # 3072 local rows in part 1
T0 = NCORE * PART0    # 25600 rows in part-0 table
T1 = NCORE * PART1    # 24576 rows in part-1 table
W = 256               # dst window width (PSUM cols)
NWIN = (NSH + W - 1) // W   # 25 windows (last one 128 wide)
NBLK = NSH // P       # 49 node blocks for the linear phase
CSH = C // NCORE      # 12500 candidates per core
GCALL = 2048          # max idxs per dma_gather call
CCH = 512             # candidate MLP chunk
BN_EPS = 1e-5
SLOPE = 0.01
F32 = mybir.dt.float32
BF16 = mybir.dt.bfloat16
F8 = mybir.dt.float8e4
I16 = mybir.dt.int16
AF = mybir.ActivationFunctionType
ALU = mybir.AluOpType
BF = ml_dtypes.bfloat16
F8NP = ml_dtypes.float8_e4m3fn


def _wrap16(idx_lin):
    """[n] -> [128, n/16] int16 in the dma_gather wrapped+replicated layout."""
    n = idx_lin.shape[0]
    assert n % 16 == 0
    w = idx_lin.reshape(n // 16, 16).T.astype(np.int16)
    return np.tile(w, (8, 1)).copy()


def _win_width(w):
    return min(W, NSH - w * W)


def _node_part(g):
    """Global node id -> (part, int16 table index) under the shard-half
    permuted table layout [part0: 8 x rows 0:3200 | part1: 8 x rows 3200:]."""
    k, r = g // NSH, g % NSH
    p = (r >= PART0).astype(np.int64)
    idx = np.where(p == 0, k * PART0 + r, k * PART1 + (r - PART0))
    return p, idx


def _prep_edges(src, dst):
    """Uniform per-core window/run/chunk schedule + per-core idx and one-hot."""
    core = dst // NSH
    winl = (dst - core * NSH) // W
    dstloc = (dst - core * NSH) - winl * W
    half, tidx = _node_part(src)
    key = (core * NWIN + winl) * 2 + half
    order = np.argsort(key, kind="stable")
    cnt = np.bincount(key, minlength=NCORE * NWIN * 2).reshape(NCORE, NWIN, 2)
    nch_u = (-(-cnt // P)).max(axis=0)        # [NWIN, 2] uniform chunk counts
    # global chunk order: w0 LO chunks, w0 HI, w1 LO, ...
    win_ch0 = np.zeros(NWIN + 1, np.int64)
    np.cumsum(nch_u.sum(axis=1), out=win_ch0[1:])
    totch = int(win_ch0[-1])
    nslot = totch * P
    # gather calls (uniform): per (w, half) run split into balanced pieces
    # of ~CTGT chunks so the 4 SWDGE queues stay evenly loaded
    CTGT = 8
    calls = {}  # (w, s) -> list of (slot_start, n_idx, chunk0)
    for w in range(NWIN):
        c0 = int(win_ch0[w])
        for s in (0, 1):
            nch = int(nch_u[w, s])
            cl = []
            if nch > 0:
                npiece = -(-nch // CTGT)
                base, rem = divmod(nch, npiece)
                cc = c0
                for i in range(npiece):
                    sz = base + (1 if i < rem else 0)
                    cl.append((cc * P, sz * P, cc))
                    cc += sz
            calls[(w, s)] = cl
            c0 += nch
    bstart = np.zeros(NCORE * NWIN * 2 + 1, np.int64)
    np.cumsum(np.bincount(key, minlength=NCORE * NWIN * 2), out=bstart[1:])
    gidx = np.zeros((NCORE, nslot), np.int16)
    ohm = np.zeros((NCORE, P, totch * W), np.uint8)  # fp8 bits (1.0 = 0x38)
    ONE = np.float32(1.0).astype(F8NP).view(np.uint8)
    for k in range(NCORE):
        for w in range(NWIN):
            pos = int(win_ch0[w]) * P
            for s in (0, 1):
                b = (k * NWIN + w) * 2 + s
                sl = order[bstart[b]:bstart[b + 1]]
                n = len(sl)
                gidx[k, pos:pos + n] = tidx[sl].astype(np.int16)
                slots = pos + np.arange(n)
                ohm[k, slots % P, (slots // P) * W + dstloc[sl]] = ONE
                pos += int(nch_u[w, s]) * P
    meta = dict(nch_u=nch_u, win_ch0=win_ch0, totch=totch, nslot=nslot,
                calls=calls)
    data = [dict(gidx=_wrap16(gidx[k]), ohm=ohm[k].view(F8NP)) for k in range(NCORE)]
    return meta, data


def _prep_cands(cand_u, cand_v, cand_feat):
    """Shard candidates, group by (u_half, v_half), pad groups to 128."""
    gch = np.zeros((NCORE, 4), np.int64)
    groups = [[None] * 4 for _ in range(NCORE)]
    for k in range(NCORE):
        ids = np.arange(k * CSH, (k + 1) * CSH)
        pu, _ = _node_part(cand_u[ids])
        pv, _ = _node_part(cand_v[ids])
        g = pu * 2 + pv
        for gi in range(4):
            groups[k][gi] = ids[g == gi]
            gch[k, gi] = -(-len(groups[k][gi]) // P)
    gch_u = gch.max(axis=0)
    goff = np.zeros(5, np.int64)
    np.cumsum(gch_u * P, out=goff[1:])
    cslot = int(goff[4])                       # 128-granular
    cu = np.zeros((NCORE, cslot), np.int16)
    cv = np.zeros((NCORE, cslot), np.int16)
    ft = np.zeros((NCORE, cslot), BF)
    mask = np.full((NCORE, cslot), -1e30, np.float32).astype(BF)
    slotmap = np.full((NCORE, cslot), -1, np.int64)
    for k in range(NCORE):
        for gi in range(4):
            ids = groups[k][gi]
            n = len(ids)
            p0 = int(goff[gi])
            _, uix = _node_part(cand_u[ids])
            _, vix = _node_part(cand_v[ids])
            cu[k, p0:p0 + n] = uix.astype(np.int16)
            cv[k, p0:p0 + n] = vix.astype(np.int16)
            ft[k, p0:p0 + n] = cand_feat[ids, 0].astype(BF)
            mask[k, p0:p0 + n] = 0.0
            slotmap[k, p0:p0 + n] = ids
    # u calls: groups 0-1 (uh=0) then 2-3 (uh=1); v calls per group;
    # balanced ~8-chunk pieces
    CTGT = 8

    def _split(lo, hi, s, out):
        nch = (hi - lo) // P
        if nch <= 0:
            return
        npiece = -(-nch // CTGT)
        base, rem = divmod(nch, npiece)
        p = lo
        for i in range(npiece):
            sz = (base + (1 if i < rem else 0)) * P
            out.append((p, sz, s))
            p += sz

    ucalls, vcalls = [], []
    _split(0, int(goff[2]), 0, ucalls)
    _split(int(goff[2]), int(goff[4]), 1, ucalls)
    for gi in range(4):
        _split(int(goff[gi]), int(goff[gi + 1]), gi % 2, vcalls)
    meta = dict(cslot=cslot, ucalls=ucalls, vcalls=vcalls)
    data = [dict(cu=_wrap16(cu[k]), cv=_wrap16(cv[k]), feat=ft[k][None, :],
                 mask=mask[k][None, :], slotmap=slotmap[k]) for k in range(NCORE)]
    return meta, data


def _build_nc(em, cm):
    nc = bacc.Bacc("TRN2", target_bir_lowering=False, debug=False,
                   num_devices=NCORE, num_swdge_queues=4)
    TOTCH, NSLOT = em["totch"], em["nslot"]
    CSLOT = cm["cslot"]
    NCC = -(-CSLOT // CCH)                  # candidate MLP chunk groups
    YCOLS = NCORE * CSLOT // P              # yfull viewed as [128, YCOLS]

    xb = nc.dram_tensor("xb", [NTOT, D], BF16, kind="ExternalInput")
    xT = nc.dram_tensor("xT", [P, NSH], BF16, kind="ExternalInput")
    gidx = nc.dram_tensor("gidx", [P, NSLOT // 16], I16, kind="ExternalInput")
    ohm = nc.dram_tensor("ohm", [P, TOTCH * W], F8, kind="ExternalInput")
    invd = nc.dram_tensor("invd", [P, NSH], BF16, kind="ExternalInput")
    wself = [nc.dram_tensor(f"wself{l}", [D, D], BF16, kind="ExternalInput") for l in range(2)]
    wneigh = [nc.dram_tensor(f"wneigh{l}", [D, D], BF16, kind="ExternalInput") for l in range(2)]
    crow = [nc.dram_tensor(f"crow{l}", [1, D], BF16, kind="ExternalInput") for l in range(2)]
    identb = nc.dram_tensor("identb", [P, P], BF16, kind="ExternalInput")
    onesr = nc.dram_tensor("onesr", [1, P], BF16, kind="ExternalInput")
    onesc = nc.dram_tensor("onesc", [1, CCH], BF16, kind="ExternalInput")
    amat = nc.dram_tensor("amat", [D, 64], BF16, kind="ExternalInput")
    bmat = nc.dram_tensor("bmat", [D, 64], BF16, kind="ExternalInput")
    mw0r = nc.dram_tensor("mw0r", [1, 64], BF16, kind="ExternalInput")
    mb0r = nc.dram_tensor("mb0r", [1, 64], BF16, kind="ExternalInput")
    mw1 = nc.dram_tensor("mw1", [64, 64], BF16, kind="ExternalInput")
    mb1r = nc.dram_tensor("mb1r", [1, 64], BF16, kind="ExternalInput")
    mw2 = nc.dram_tensor("mw2c", [64, 1], BF16, kind="ExternalInput")
    mb2r = nc.dram_tensor("mb2r", [1, 1], BF16, kind="ExternalInput")
    cu = nc.dram_tensor("cu", [P, CSLOT // 16], I16, kind="ExternalInput")
    cv = nc.dram_tensor("cv", [P, CSLOT // 16], I16, kind="ExternalInput")
    featr = nc.dram_tensor("featr", [1, CSLOT], BF16, kind="ExternalInput")
    maskr = nc.dram_tensor("maskr", [1, CSLOT], BF16, kind="ExternalInput")

    y_out = nc.dram_tensor("y_out", [P, YCOLS], F32, kind="ExternalOutput")
    p_out = nc.dram_tensor("p_out", [P, YCOLS], F32, kind="ExternalOutput")

    hshA = nc.dram_tensor("hshA", [PART0, D], BF16, kind="Internal")
    hshB = nc.dram_tensor("hshB", [PART1, D], BF16, kind="Internal")
    hfull0 = nc.dram_tensor("hfull0", [T0, D], BF16, kind="Internal",
                            addr_space="Shared")
    hfull1 = nc.dram_tensor("hfull1", [T1, D], BF16, kind="Internal",
                            addr_space="Shared")
    gqshA = nc.dram_tensor("gqshA", [PART0, D], BF16, kind="Internal")
    gqshB = nc.dram_tensor("gqshB", [PART1, D], BF16, kind="Internal")
    gqfull0 = nc.dram_tensor("gqfull0", [T0, D], BF16, kind="Internal",
                             addr_space="Shared")
    gqfull1 = nc.dram_tensor("gqfull1", [T1, D], BF16, kind="Internal",
                             addr_space="Shared")
    ysh = nc.dram_tensor("ysh", [1, CSLOT], F32, kind="Internal")
    yfull = nc.dram_tensor("yfull", [NCORE, CSLOT], F32, kind="Internal",
                           addr_space="Shared")

    rg = [list(range(NCORE))]
    nch_u, win_ch0, calls = em["nch_u"], em["win_ch0"], em["calls"]

    with tile.TileContext(nc) as tc:
        with (
            tc.tile_pool(name="const", bufs=1) as cp,
            tc.tile_pool(name="big", bufs=1) as bp,
            tc.tile_pool(name="msgs", bufs=9) as mp,
            tc.tile_pool(name="ohp", bufs=4) as op_,
            tc.tile_pool(name="wrk", bufs=4) as wp,
            tc.tile_pool(name="frows", bufs=3) as fp_,
            tc.tile_pool(name="gup", bufs=3) as gup,
            tc.tile_pool(name="gvp", bufs=3) as gvp,
            tc.tile_pool(name="z1p", bufs=2) as z1p,
        ):
            nc.gpsimd.load_library(library_config.mlp)

            def load(pool, t, shape=None):
                tl = pool.tile(shape or list(t.shape), t.dtype, tag=t.name)
                nc.sync.dma_start(tl[:], t[:])
                return tl

            gidx_t = load(cp, gidx)
            invd_t = load(cp, invd)
            identb_t = load(cp, identb)
            onesr_t = load(cp, onesr)
            onesc_t = load(cp, onesc)
            wself_t = [load(cp, t) for t in wself]
            wneigh_t = [load(cp, t) for t in wneigh]
            crow_t = [load(cp, t) for t in crow]
            amat_t = load(cp, amat)
            bmat_t = load(cp, bmat)
            mw0r_t = load(cp, mw0r)
            mb0r_t = load(cp, mb0r)
            mw1_t = load(cp, mw1)
            mb1r_t = load(cp, mb1r)
            mw2_t = load(cp, mw2)
            mb2r_t = load(cp, mb2r)
            cu_t = load(cp, cu)
            cv_t = load(cp, cv)
            featr_t = load(cp, featr)

            curT = bp.tile([P, NSH], BF16, tag="curT")
            nxtT = bp.tile([P, NSH], BF16, tag="nxtT")
            aggrb = bp.tile([P, NSH], BF16, tag="aggrb")
            nc.sync.dma_start(curT[:], xT[:])

            qrr = [0]

            def next_q():
                q = qrr[0]
                qrr[0] = (q + 1) % 4
                return q

            ctxA = tc.tile_pool(name="ps_w", bufs=3, space="PSUM")
            ps_w = ctxA.__enter__()
            ctxB = tc.tile_pool(name="ps_l", bufs=2, space="PSUM")
            ps_l = ctxB.__enter__()
            ctxC = tc.tile_pool(name="ps_t", bufs=1, space="PSUM")
            ps_t = ctxC.__enter__()

            def linear_block(layer, b):
                ph = ps_l.tile([P, P], F32, tag="ph")
                nc.tensor.matmul(ph[:], lhsT=aggrb[:, b * P:(b + 1) * P],
                                 rhs=wneigh_t[layer][:], start=True, stop=False)
                nc.tensor.matmul(ph[:], lhsT=curT[0][:, b * P:(b + 1) * P],
                                 rhs=wself_t[layer][:], start=False, stop=False)
                nc.tensor.matmul(ph[:], lhsT=onesr_t[:], rhs=crow_t[layer][:],
                                 start=False, stop=True)
                tmp = wp.tile([P, P], BF16, tag="tmp")
                nc.vector.tensor_scalar(tmp[:], ph[:], SLOPE, None, ALU.mult)
                ht = wp.tile([P, P], BF16, tag="ht")
                nc.vector.tensor_tensor(ht[:], ph[:], tmp[:], ALU.max)
                if layer == 0:
                    if b < PART0 // P:
                        nc.sync.dma_start(hshA[b * P:(b + 1) * P, :], ht[:])
                    else:
                        b1 = b - PART0 // P
                        nc.sync.dma_start(hshB[b1 * P:(b1 + 1) * P, :], ht[:])
                pt = ps_t.tile([P, P], BF16, tag="pt")
                nc.tensor.transpose(pt[:], ht[:], identb_t[:])
                nc.scalar.activation(nxtT[0][:, b * P:(b + 1) * P], pt[:],
                                     AF.Copy)
                if layer == 0:
                    if b == NBLK - 1:
                        nc.gpsimd.collective_compute(
                            "AllGather", ALU.bypass, replica_groups=rg,
                            ins=[hshB[:].opt()], outs=[hfull1[:].opt()])
                    if b == PART0 // P - 2:
                        nc.gpsimd.collective_compute(
                            "AllGather", ALU.bypass, replica_groups=rg,
                            ins=[hshA[:].opt()], outs=[hfull0[:].opt()])
                else:
                    pg = ps_l.tile([P, P], F32, tag="ph")
                    nc.tensor.matmul(pg[:, 0:64],
                                     lhsT=nxtT[0][:, b * P:(b + 1) * P],
                                     rhs=amat_t[:], start=True, stop=True)
                    nc.tensor.matmul(pg[:, 64:128],
                                     lhsT=nxtT[0][:, b * P:(b + 1) * P],
                                     rhs=bmat_t[:], start=True, stop=False)
                    nc.tensor.matmul(pg[:, 64:128], lhsT=onesr_t[:],
                                     rhs=mb0r_t[:], start=False, stop=True)
                    gq = wp.tile([P, P], BF16, tag="gq")
                    nc.scalar.activation(gq[:], pg[:], AF.Copy)
                    if b < PART0 // P:
                        nc.sync.dma_start(gqshA[b * P:(b + 1) * P, :], gq[:])
                    else:
                        b1 = b - PART0 // P
                        nc.sync.dma_start(gqshB[b1 * P:(b1 + 1) * P, :], gq[:])
                    if b == PART0 // P - 1:
                        nc.gpsimd.collective_compute(
                            "AllGather", ALU.bypass, replica_groups=rg,
                            ins=[gqshA[:].opt()],
                            outs=[gqfull0[:].opt()])

            def run_gathers(ps, gtab_s, cl, first, mtot):
                done = first
                for (p0, n_idx, ch0) in cl:
                    ncall = n_idx // P
                    g = mp.tile([P, GCALL // P, P], BF16, tag="g")
                    nc.gpsimd.dma_gather(
                        g[:, :ncall, :], gtab_s[:],
                        gidx_t[:, p0 // 16:(p0 + n_idx) // 16],
                        n_idx, n_idx, P, single_packet=False,
                        queue_num=next_q())
                    oht = op_.tile([P, GCALL // P, W], F8, tag="oh")
                    nc.sync.dma_start(
                        oht[:, :ncall, :],
                        ohm[:, ch0 * W:(ch0 + ncall) * W]
                        .rearrange("p (a b) -> p a b", b=W))
                    for cc in range(ncall):
                        nc.tensor.matmul(
                            ps[:], lhsT=g[:, cc, :], rhs=oht[:, cc, :],
                            start=(done == 0), stop=(done == mtot - 1))
                        done += 1

            curT, nxtT = [curT], [nxtT]
            l0_order = list(range(12, NWIN)) + list(range(0, 12))
            for layer in range(2):
                tabs = ((xb[0:T0, :], xb[T0:NTOT, :]) if layer == 0
                        else (hfull0[:], hfull1[:]))
                worder = l0_order if layer == 0 else range(NWIN)
                for w in worder:
                    ww = _win_width(w)
                    wsl = aggrb[:, w * W:w * W + ww]
                    nch0 = int(nch_u[w, 0])
                    nch1 = int(nch_u[w, 1])
                    mtot = nch0 + nch1
                    if mtot == 0:
                        nc.vector.memset(wsl, 0.0)
                    else:
                        ps = ps_w.tile([P, W], F32, tag="psw")
                        if layer == 0:
                            run_gathers(ps, tabs[0], calls[(w, 0)], 0, mtot)
                            run_gathers(ps, tabs[1], calls[(w, 1)], nch0, mtot)
                        else:
                            run_gathers(ps, tabs[1], calls[(w, 1)], 0, mtot)
                            run_gathers(ps, tabs[0], calls[(w, 0)], nch1, mtot)
                        nc.vector.tensor_tensor(
                            wsl, ps[:, :ww],
                            invd_t[:, w * W:w * W + ww], ALU.mult)
                    for b in range((w * W) // P, (w * W + ww) // P):
                        linear_block(layer, b)
                curT[0], nxtT[0] = nxtT[0], curT[0]
            nc.gpsimd.collective_compute(
                "AllGather", ALU.bypass, replica_groups=rg,
                ins=[gqshB[:].opt()], outs=[gqfull1[:].opt()])

            ctxC.__exit__(None, None, None)
            ctxB.__exit__(None, None, None)
            ctxA.__exit__(None, None, None)
            with (
                tc.tile_pool(name="psc1", bufs=2, space="PSUM") as psc1,
                tc.tile_pool(name="psc2", bufs=2, space="PSUM") as psc2,
                tc.tile_pool(name="ps_c", bufs=2, space="PSUM") as ps_c,
                tc.tile_pool(name="ps_y", bufs=1, space="PSUM") as ps_y,
            ):
                # ---- candidate gathers (slot layout, 4-queue) ----
                ucalls, vcalls = cm["ucalls"], cm["vcalls"]
                uci, vci = [], []
                for i, (s0, n, sh) in enumerate(ucalls):
                    uci += [(i, j) for j in range(n // P)]
                for i, (s0, n, sh) in enumerate(vcalls):
                    vci += [(i, j) for j in range(n // P)]
                gu_tiles = [None] * len(ucalls)
                gv_tiles = [None] * len(vcalls)
                for i in range(max(len(ucalls), len(vcalls))):
                    for (tiles, calls, pool, idx_t) in (
                            (gu_tiles, ucalls, gup, cu_t),
                            (gv_tiles, vcalls, gvp, cv_t)):
                        if i >= len(calls):
                            continue
                        (s0, n_idx, sh) = calls[i]
                        ncall = n_idx // P
                        g = pool.tile([P, GCALL // P, P], BF16, tag="g")
                        nc.gpsimd.dma_gather(
                            g[:, :ncall, :],
                            (gqfull0 if sh == 0 else gqfull1)[:],
                            idx_t[:, s0 // 16:(s0 + n_idx) // 16],
                            n_idx, n_idx, P, single_packet=False,
                            queue_num=next_q())
                        tiles[i] = g
                # ---- candidate MLP ----
                nchunk = CSLOT // P
                z1t = None
                for c in range(nchunk):
                    ui, uo = uci[c]
                    vi, vo = vci[c]
                    psf = psc1.tile([P, 64], F32, tag="psf")
                    nc.tensor.matmul(psf[:], lhsT=identb_t[:],
                                     rhs=gu_tiles[ui][:, uo, 0:64],
                                     start=True, stop=False)
                    nc.tensor.matmul(psf[:], lhsT=identb_t[:],
                                     rhs=gv_tiles[vi][:, vo, 64:128],
                                     start=False, stop=False)
                    nc.tensor.matmul(psf[:], lhsT=featr_t[:, c * P:(c + 1) * P],
                                     rhs=mw0r_t[:], start=False, stop=True)
                    zs = wp.tile([P, 64], BF16, tag="zs")
                    nc.vector.tensor_scalar(zs[:], psf[:], SLOPE, None, ALU.mult)
                    z1b = wp.tile([P, 64], BF16, tag="z1b")
                    nc.vector.tensor_tensor(z1b[:], psf[:], zs[:], ALU.max)
                    ptz = psc2.tile([64, P], BF16, tag="ptz")
                    nc.tensor.transpose(ptz[:], z1b[:], identb_t[:])
                    if c % 4 == 0:
                        z1t = z1p.tile([64, CCH], BF16, tag="z1t")
                    nc.scalar.activation(z1t[:, (c % 4) * P:(c % 4 + 1) * P],
                                         ptz[:], AF.Copy)
                    if c % 4 == 3 or c == nchunk - 1:
                        g0 = (c // 4) * CCH
                        gw = min(CCH, CSLOT - g0)
                        mr = fp_.tile([1, CCH], BF16, tag="mr")
                        nc.sync.dma_start(mr[:, :gw], maskr[:, g0:g0 + gw])
                        psz = ps_c.tile([64, CCH], F32, tag="psc")
                        nc.tensor.matmul(psz[:, :gw], lhsT=mw1_t[:],
                                         rhs=z1t[:, :gw], start=True, stop=False)
                        nc.tensor.matmul(psz[:, :gw], lhsT=mb1r_t[:],
                                         rhs=onesc_t[:, :gw],
                                         start=False, stop=True)
                        zs2 = wp.tile([64, CCH], BF16, tag="zs2")
                        nc.vector.tensor_scalar(zs2[:, :gw], psz[:, :gw],
                                                SLOPE, None, ALU.mult)
                        z2 = wp.tile([64, CCH], BF16, tag="z2")
                        nc.vector.tensor_tensor(z2[:, :gw], psz[:, :gw],
                                                zs2[:, :gw], ALU.max)
                        py = ps_y.tile([1, CCH], F32, tag="psy")
                        nc.tensor.matmul(py[:, :gw], lhsT=mw2_t[:],
                                         rhs=z2[:, :gw], start=True, stop=False)
                        nc.tensor.matmul(py[:, :gw], lhsT=mb2r_t[:],
                                         rhs=onesc_t[:, :gw],
                                         start=False, stop=True)
                        ym = fp_.tile([1, CCH], F32, tag="ym")
                        nc.vector.tensor_tensor(ym[:, :gw], py[:, :gw],
                                                mr[:, :gw], ALU.add)
                        nc.sync.dma_start(ysh[:, g0:g0 + gw], ym[:, :gw])
            nc.gpsimd.collective_compute(
                "AllGather", ALU.bypass, replica_groups=rg,
                ins=[ysh[:].opt()], outs=[yfull[:].opt()])
            # ---- softmax over yfull viewed as [128, YCOLS] ----
            yf = bp.tile([P, YCOLS], F32, tag="yf")
            nc.sync.dma_start(yf[:], yfull[:].rearrange("a b -> (a b)")
                              .rearrange("(p c) -> p c", p=P))
            nc.sync.dma_start(y_out[:], yf[:])
            rmax = wp.tile([P, 1], F32, tag="rmax")
            nc.vector.tensor_reduce(rmax[:], yf[:], mybir.AxisListType.X, ALU.max)
            gmax = wp.tile([P, 1], F32, tag="gmax")
            nc.gpsimd.partition_all_reduce(gmax[:], rmax[:], P,
                                           bass_isa.ReduceOp.max)
            ngmax = wp.tile([P, 1], F32, tag="ngmax")
            nc.vector.tensor_scalar(ngmax[:], gmax[:], -1.0, None, ALU.mult)
            ef = bp.tile([P, YCOLS], F32, tag="ef")
            se = wp.tile([P, 1], F32, tag="se")
            nc.scalar.activation(ef[:], yf[:], AF.Exp, bias=ngmax[:, 0:1],
                                 accum_out=se[:])
            stot = wp.tile([P, 1], F32, tag="stot")
            nc.gpsimd.partition_all_reduce(stot[:], se[:], P,
                                           bass_isa.ReduceOp.add)
            invs = wp.tile([P, 1], F32, tag="invs")
            nc.vector.reciprocal(invs[:], stot[:])
            pf = bp.tile([P, YCOLS], F32, tag="pf")
            nc.vector.tensor_scalar(pf[:], ef[:], invs[:, 0:1], None, ALU.mult)
            nc.sync.dma_start(p_out[:], pf[:])
    nc.compile()
    return nc


def kernel(x, src, dst, cand_u, cand_v, cand_feat,
           w_self0, w_neigh0, b0, gamma0, beta0, rm0, rv0,
           w_self1, w_neigh1, b1, gamma1, beta1, rm1, rv1,
           mw0, mb0, mw1, mb1, mw2, mb2):
    x = np.asarray(x, np.float32)
    src = np.asarray(src, np.int64)
    dst = np.asarray(dst, np.int64)
    cand_u = np.asarray(cand_u, np.int64)
    cand_v = np.asarray(cand_v, np.int64)
    cand_feat = np.asarray(cand_feat, np.float32)

    deg = np.bincount(dst, minlength=N).astype(np.float32)
    invdeg = 1.0 / np.maximum(deg, 1.0)
    em, edata = _prep_edges(src, dst)
    cm, cdata = _prep_cands(cand_u, cand_v, cand_feat)

    xpad = np.zeros((NTOT, D), np.float32)
    xpad[:N] = x
    invdp = np.zeros(NTOT, np.float32)
    invdp[:N] = invdeg
    # permuted gather table: [part0: 8 x rows 0:3200 | part1: 8 x rows 3200:]
    gids = np.arange(NTOT)
    pp, tix = _node_part(gids)
    xperm = np.zeros((NTOT, D), np.float32)
    xperm[np.where(pp == 0, tix, T0 + tix)] = xpad[gids]

    com = {
        "xb": xperm.astype(BF),
        "identb": np.eye(P, dtype=np.float32).astype(BF),
        "onesr": np.ones((1, P), BF),
        "onesc": np.ones((1, CCH), BF),
    }
    for l, (ws, wn, b, ga, be, rme, rve) in enumerate(
        ((w_self0, w_neigh0, b0, gamma0, beta0, rm0, rv0),
         (w_self1, w_neigh1, b1, gamma1, beta1, rm1, rv1))):
        a = (np.asarray(ga) / np.sqrt(np.asarray(rve) + BN_EPS)).astype(np.float32)
        com[f"wself{l}"] = (np.asarray(ws) * a[None, :]).astype(BF)
        com[f"wneigh{l}"] = (np.asarray(wn) * a[None, :]).astype(BF)
        com[f"crow{l}"] = (a * (np.asarray(b) - np.asarray(rme)) + np.asarray(be)).astype(BF)[None, :]
    com["amat"] = np.asarray(mw0[0:128], np.float32).astype(BF)
    com["bmat"] = np.asarray(mw0[128:256], np.float32).astype(BF)
    com["mw0r"] = np.asarray(mw0[256], np.float32).astype(BF)[None, :]
    com["mb0r"] = np.asarray(mb0, np.float32).astype(BF)[None, :]
    com["mw1"] = np.asarray(mw1, np.float32).astype(BF)
    com["mb1r"] = np.asarray(mb1, np.float32).astype(BF)[None, :]
    com["mw2c"] = np.asarray(mw2, np.float32).astype(BF)
    com["mb2r"] = np.asarray(mb2, np.float32).reshape(1, 1).astype(BF)

    nc = _build_nc(em, cm)
    in_maps = []
    for k in range(NCORE):
        m = dict(com)
        m["xT"] = xpad[k * NSH:(k + 1) * NSH].T.astype(BF).copy()
        m["invd"] = np.tile(invdp[k * NSH:(k + 1) * NSH].astype(BF), (P, 1))
        m["gidx"] = edata[k]["gidx"]
        m["ohm"] = edata[k]["ohm"]
        m["cu"] = cdata[k]["cu"]
        m["cv"] = cdata[k]["cv"]
        m["featr"] = cdata[k]["feat"]
        m["maskr"] = cdata[k]["mask"]
        in_maps.append(m)
    trace = bool(os.environ.get("KERNEL_TRACE"))
    if trace:
        import types
        if "antenv.axon_hooks" not in sys.modules:
            try:
                import antenv
                from trn_agent_boot.trn_boot import _ntff_profile_via_ctypes
                mod = types.ModuleType("antenv.axon_hooks")
                hook = [_ntff_profile_via_ctypes("/opt/axon/libaxon_pjrt.so")]
                mod.set_axon_ntff_profile_hook = lambda h: hook.__setitem__(0, h)
                mod.get_axon_ntff_profile_hook = lambda: hook[0]
                sys.modules["antenv.axon_hooks"] = mod
                antenv.axon_hooks = mod
            except Exception:
                trace = False
    res = run_bass_kernel_spmd(nc, in_maps, core_ids=list(range(NCORE)),
                               trace=trace,
                               tmpdir=os.environ.get("KERNEL_TRACE_DIR"))
    if trace and res.exec_time_ns is not None:
        print(f"HW exec time: {res.exec_time_ns} ns")
    y_all = np.zeros(C, np.float32)
    p_all = np.zeros(C, np.float32)
    cslot = cm["cslot"]
    y_lin = res.results[0]["y_out"].ravel()   # flat order: core, slot
    p_lin = res.results[0]["p_out"].ravel()
    for k in range(NCORE):
        sm = cdata[k]["slotmap"]
        valid = sm >= 0
        j = np.nonzero(valid)[0]
        y_all[sm[valid]] = y_lin[k * cslot + j]
        p_all[sm[valid]] = p_lin[k * cslot + j]
    return y_all[:, None], p_all[:, None]


# revision 18
# speedup vs baseline: 1.1216x; 1.0008x over previous
"""Trainium2 Bass kernel for nn_PolicyNetwork3 (2-layer GraphSAGE + edge-MLP).

v2 design (8 NeuronCores, SPMD single NEFF):
- dst-sharded aggregation; core k owns node block [6272k, 6272k+6272).
- Edge messages gathered per-edge from bf16 HBM row tables via 4-queue
  round-robin dma_gather (descriptor generation parallelizes across the
  SWDGE queues; ~2.4ns/row vs 8ns serialized).
- segment-sum per 256-dst window via one-hot matmuls (bf16 gathered rows x
  fp8 0/1 one-hot streamed from HBM) accumulating in PSUM across the LO/HI
  half-table runs; per-window PSUM->SBUF copy fuses the 1/deg scale.
- BN folded into SAGE weights on host; linear+leaky per 128-node block;
  h shards exchanged with bf16 AllGather.
- candidate MLP: g/q tables per node; transposed candidate gathers put
  features on partitions so the whole MLP runs as 512-wide PE matmuls
  (identity-matmul adds, K=1 bias rows); global softmax on-device.
"""

import os
import sys

sys.path.insert(0, "/opt/trn_rl_repo")
sys.path.insert(0, "/root/.axon_site")

import numpy as np
import ml_dtypes

import concourse.bacc as bacc
import concourse.bass as bass
import concourse.bass_isa as bass_isa
import concourse.mybir as mybir
import concourse.tile as tile
from concourse import library_config
from concourse.bass_utils import run_bass_kernel_spmd

P = 128
N, E, C = 50000, 800000, 100000
D = 128
NCORE = 8
NSH = 6272            # nodes per core shard
NTOT = NSH * NCORE    # 50176 padded node table
HALF = NTOT // 2      # 25088 rows per gather-table half
PART0 = 3200          # local rows in table part 0 (25 blocks)
PART1 = NSH - PART0   # 3072 local rows in part 1
T0 = NCORE * PART0    # 25600 rows in part-0 table
T1 = NCORE * PART1    # 24576 rows in part-1 table
W = 256               # dst window width (PSUM cols)
NWIN = (NSH + W - 1) // W   # 25 windows (last one 128 wide)
NBLK = NSH // P       # 49 node blocks for the linear phase
CSH = C // NCORE      # 12500 candidates per core
GCALL = 2048          # max idxs per dma_gather call
CCH = 512             # candidate MLP chunk
BN_EPS = 1e-5
SLOPE = 0.01
F32 = mybir.dt.float32
BF16 = mybir.dt.bfloat16
F8 = mybir.dt.float8e4
I16 = mybir.dt.int16
AF = mybir.ActivationFunctionType
ALU = mybir.AluOpType
BF = ml_dtypes.bfloat16
F8NP = ml_dtypes.float8_e4m3fn


def _wrap16(idx_lin):
    """[n] -> [128, n/16] int16 in the dma_gather wrapped+replicated layout."""
    n = idx_lin.shape[0]
    assert n % 16 == 0
    w = idx_lin.reshape(n // 16, 16).T.astype(np.int16)
    return np.tile(w, (8, 1)).copy()


def _win_width(w):
    return min(W, NSH - w * W)


def _node_part(g):
    """Global node id -> (part, int16 table index) under the shard-half
    permuted table layout [part0: 8 x rows 0:3200 | part1: 8 x rows 3200:]."""
    k, r = g // NSH, g % NSH
    p = (r >= PART0).astype(np.int64)
    idx = np.where(p == 0, k * PART0 + r, k * PART1 + (r - PART0))
    return p, idx


def _prep_edges(src, dst):
    """Uniform per-core window/run/chunk schedule + per-core idx and one-hot."""
    core = dst // NSH
    winl = (dst - core * NSH) // W
    dstloc = (dst - core * NSH) - winl * W
    half, tidx = _node_part(src)
    key = (core * NWIN + winl) * 2 + half
    order = np.argsort(key, kind="stable")
    cnt = np.bincount(key, minlength=NCORE * NWIN * 2).reshape(NCORE, NWIN, 2)
    nch_u = (-(-cnt // P)).max(axis=0)        # [NWIN, 2] uniform chunk counts
    # global chunk order: w0 LO chunks, w0 HI, w1 LO, ...
    win_ch0 = np.zeros(NWIN + 1, np.int64)
    np.cumsum(nch_u.sum(axis=1), out=win_ch0[1:])
    totch = int(win_ch0[-1])
    nslot = totch * P
    # gather calls (uniform): per (w, half) run split into balanced pieces
    # of ~CTGT chunks so the 4 SWDGE queues stay evenly loaded
    CTGT = 16
    calls = {}  # (w, s) -> list of (slot_start, n_idx, chunk0)
    for w in range(NWIN):
        c0 = int(win_ch0[w])
        for s in (0, 1):
            nch = int(nch_u[w, s])
            cl = []
            if nch > 0:
                npiece = -(-nch // CTGT)
                base, rem = divmod(nch, npiece)
                cc = c0
                for i in range(npiece):
                    sz = base + (1 if i < rem else 0)
                    cl.append((cc * P, sz * P, cc))
                    cc += sz
            calls[(w, s)] = cl
            c0 += nch
    bstart = np.zeros(NCORE * NWIN * 2 + 1, np.int64)
    np.cumsum(np.bincount(key, minlength=NCORE * NWIN * 2), out=bstart[1:])
    gidx = np.zeros((NCORE, nslot), np.int16)
    ohm = np.zeros((NCORE, P, totch * W), np.uint8)  # fp8 bits (1.0 = 0x38)
    ONE = np.float32(1.0).astype(F8NP).view(np.uint8)
    for k in range(NCORE):
        for w in range(NWIN):
            pos = int(win_ch0[w]) * P
            for s in (0, 1):
                b = (k * NWIN + w) * 2 + s
                sl = order[bstart[b]:bstart[b + 1]]
                n = len(sl)
                gidx[k, pos:pos + n] = tidx[sl].astype(np.int16)
                slots = pos + np.arange(n)
                ohm[k, slots % P, (slots // P) * W + dstloc[sl]] = ONE
                pos += int(nch_u[w, s]) * P
    meta = dict(nch_u=nch_u, win_ch0=win_ch0, totch=totch, nslot=nslot,
                calls=calls)
    data = [dict(gidx=_wrap16(gidx[k]), ohm=ohm[k].view(F8NP)) for k in range(NCORE)]
    return meta, data


def _prep_cands(cand_u, cand_v, cand_feat):
    """Shard candidates, group by (u_half, v_half), pad groups to 128."""
    gch = np.zeros((NCORE, 4), np.int64)
    groups = [[None] * 4 for _ in range(NCORE)]
    for k in range(NCORE):
        ids = np.arange(k * CSH, (k + 1) * CSH)
        pu, _ = _node_part(cand_u[ids])
        pv, _ = _node_part(cand_v[ids])
        g = pu * 2 + pv
        for gi in range(4):
            groups[k][gi] = ids[g == gi]
            gch[k, gi] = -(-len(groups[k][gi]) // P)
    gch_u = gch.max(axis=0)
    goff = np.zeros(5, np.int64)
    np.cumsum(gch_u * P, out=goff[1:])
    cslot = int(goff[4])                       # 128-granular
    cu = np.zeros((NCORE, cslot), np.int16)
    cv = np.zeros((NCORE, cslot), np.int16)
    ft = np.zeros((NCORE, cslot), BF)
    mask = np.full((NCORE, cslot), -1e30, np.float32).astype(BF)
    slotmap = np.full((NCORE, cslot), -1, np.int64)
    for k in range(NCORE):
        for gi in range(4):
            ids = groups[k][gi]
            n = len(ids)
            p0 = int(goff[gi])
            _, uix = _node_part(cand_u[ids])
            _, vix = _node_part(cand_v[ids])
            cu[k, p0:p0 + n] = uix.astype(np.int16)
            cv[k, p0:p0 + n] = vix.astype(np.int16)
            ft[k, p0:p0 + n] = cand_feat[ids, 0].astype(BF)
            mask[k, p0:p0 + n] = 0.0
            slotmap[k, p0:p0 + n] = ids
    # u calls: groups 0-1 (uh=0) then 2-3 (uh=1); v calls per group;
    # balanced ~8-chunk pieces
    CTGT = 16

    def _split(lo, hi, s, out):
        nch = (hi - lo) // P
        if nch <= 0:
            return
        npiece = -(-nch // CTGT)
        base, rem = divmod(nch, npiece)
        p = lo
        for i in range(npiece):
            sz = (base + (1 if i < rem else 0)) * P
            out.append((p, sz, s))
            p += sz

    ucalls, vcalls = [], []
    _split(0, int(goff[2]), 0, ucalls)
    _split(int(goff[2]), int(goff[4]), 1, ucalls)
    for gi in range(4):
        _split(int(goff[gi]), int(goff[gi + 1]), gi % 2, vcalls)
    meta = dict(cslot=cslot, ucalls=ucalls, vcalls=vcalls)
    data = [dict(cu=_wrap16(cu[k]), cv=_wrap16(cv[k]), feat=ft[k][None, :],
                 mask=mask[k][None, :], slotmap=slotmap[k]) for k in range(NCORE)]
    return meta, data


def _build_nc(em, cm):
    nc = bacc.Bacc("TRN2", target_bir_lowering=False, debug=False,
                   num_devices=NCORE, num_swdge_queues=4)
    TOTCH, NSLOT = em["totch"], em["nslot"]
    CSLOT = cm["cslot"]
    NCC = -(-CSLOT // CCH)                  # candidate MLP chunk groups
    YCOLS = NCORE * CSLOT // P              # yfull viewed as [128, YCOLS]

    xb = nc.dram_tensor("xb", [NTOT, D], BF16, kind="ExternalInput")
    xT = nc.dram_tensor("xT", [P, NSH], BF16, kind="ExternalInput")
    gidx = nc.dram_tensor("gidx", [P, NSLOT // 16], I16, kind="ExternalInput")
    ohm = nc.dram_tensor("ohm", [P, TOTCH * W], F8, kind="ExternalInput")
    invd = nc.dram_tensor("invd", [P, NSH], BF16, kind="ExternalInput")
    wself = [nc.dram_tensor(f"wself{l}", [D, D], BF16, kind="ExternalInput") for l in range(2)]
    wneigh = [nc.dram_tensor(f"wneigh{l}", [D, D], BF16, kind="ExternalInput") for l in range(2)]
    crow = [nc.dram_tensor(f"crow{l}", [1, D], BF16, kind="ExternalInput") for l in range(2)]
    identb = nc.dram_tensor("identb", [P, P], BF16, kind="ExternalInput")
    onesr = nc.dram_tensor("onesr", [1, P], BF16, kind="ExternalInput")
    onesc = nc.dram_tensor("onesc", [1, CCH], BF16, kind="ExternalInput")
    amat = nc.dram_tensor("amat", [D, 64], BF16, kind="ExternalInput")
    bmat = nc.dram_tensor("bmat", [D, 64], BF16, kind="ExternalInput")
    mw0r = nc.dram_tensor("mw0r", [1, 64], BF16, kind="ExternalInput")
    mb0r = nc.dram_tensor("mb0r", [1, 64], BF16, kind="ExternalInput")
    mw1 = nc.dram_tensor("mw1", [64, 64], BF16, kind="ExternalInput")
    mb1r = nc.dram_tensor("mb1r", [1, 64], BF16, kind="ExternalInput")
    mw2 = nc.dram_tensor("mw2c", [64, 1], BF16, kind="ExternalInput")
    mb2r = nc.dram_tensor("mb2r", [1, 1], BF16, kind="ExternalInput")
    cu = nc.dram_tensor("cu", [P, CSLOT // 16], I16, kind="ExternalInput")
    cv = nc.dram_tensor("cv", [P, CSLOT // 16], I16, kind="ExternalInput")
    featr = nc.dram_tensor("featr", [1, CSLOT], BF16, kind="ExternalInput")
    maskr = nc.dram_tensor("maskr", [1, CSLOT], BF16, kind="ExternalInput")

    y_out = nc.dram_tensor("y_out", [P, YCOLS], F32, kind="ExternalOutput")
    p_out = nc.dram_tensor("p_out", [P, YCOLS], F32, kind="ExternalOutput")

    hshA = nc.dram_tensor("hshA", [PART0, D], BF16, kind="Internal")
    hshB = nc.dram_tensor("hshB", [PART1, D], BF16, kind="Internal")
    hfull0 = nc.dram_tensor("hfull0", [T0, D], BF16, kind="Internal",
                            addr_space="Shared")
    hfull1 = nc.dram_tensor("hfull1", [T1, D], BF16, kind="Internal",
                            addr_space="Shared")
    gqshA = nc.dram_tensor("gqshA", [PART0, D], BF16, kind="Internal")
    gqshB = nc.dram_tensor("gqshB", [PART1, D], BF16, kind="Internal")
    gqfull0 = nc.dram_tensor("gqfull0", [T0, D], BF16, kind="Internal",
                             addr_space="Shared")
    gqfull1 = nc.dram_tensor("gqfull1", [T1, D], BF16, kind="Internal",
                             addr_space="Shared")
    ysh = nc.dram_tensor("ysh", [1, CSLOT], F32, kind="Internal")
    yfull = nc.dram_tensor("yfull", [NCORE, CSLOT], F32, kind="Internal",
                           addr_space="Shared")

    rg = [list(range(NCORE))]
    nch_u, win_ch0, calls = em["nch_u"], em["win_ch0"], em["calls"]

    with tile.TileContext(nc) as tc:
        with (
            tc.tile_pool(name="const", bufs=1) as cp,
            tc.tile_pool(name="big", bufs=1) as bp,
            tc.tile_pool(name="msgs", bufs=12) as mp,
            tc.tile_pool(name="ohp", bufs=5) as op_,
            tc.tile_pool(name="wrk", bufs=4) as wp,
            tc.tile_pool(name="frows", bufs=3) as fp_,
            tc.tile_pool(name="gup", bufs=4) as gup,
            tc.tile_pool(name="gvp", bufs=4) as gvp,
            tc.tile_pool(name="z1p", bufs=2) as z1p,
        ):
            nc.gpsimd.load_library(library_config.mlp)

            def load(pool, t, shape=None):
                tl = pool.tile(shape or list(t.shape), t.dtype, tag=t.name)
                nc.sync.dma_start(tl[:], t[:])
                return tl

            gidx_t = load(cp, gidx)
            invd_t = load(cp, invd)
            identb_t = load(cp, identb)
            onesr_t = load(cp, onesr)
            onesc_t = load(cp, onesc)
            wself_t = [load(cp, t) for t in wself]
            wneigh_t = [load(cp, t) for t in wneigh]
            crow_t = [load(cp, t) for t in crow]
            amat_t = load(cp, amat)
            bmat_t = load(cp, bmat)
            mw0r_t = load(cp, mw0r)
            mb0r_t = load(cp, mb0r)
            mw1_t = load(cp, mw1)
            mb1r_t = load(cp, mb1r)
            mw2_t = load(cp, mw2)
            mb2r_t = load(cp, mb2r)
            cu_t = load(cp, cu)
            cv_t = load(cp, cv)

            curT = bp.tile([P, NSH], BF16, tag="curT")
            nxtT = bp.tile([P, NSH], BF16, tag="nxtT")
            aggrb = bp.tile([P, NSH], BF16, tag="aggrb")
            nc.sync.dma_start(curT[:], xT[:])

            qrr = [0]

            def next_q():
                q = qrr[0]
                qrr[0] = (q + 1) % 4
                return q

            ctxA = tc.tile_pool(name="ps_w", bufs=3, space="PSUM")
            ps_w = ctxA.__enter__()
            ctxB = tc.tile_pool(name="ps_l", bufs=2, space="PSUM")
            ps_l = ctxB.__enter__()
            ctxC = tc.tile_pool(name="ps_t", bufs=1, space="PSUM")
            ps_t = ctxC.__enter__()

            def linear_block(layer, b):
                ph = ps_l.tile([P, P], F32, tag="ph")
                nc.tensor.matmul(ph[:], lhsT=aggrb[:, b * P:(b + 1) * P],
                                 rhs=wneigh_t[layer][:], start=True, stop=False)
                nc.tensor.matmul(ph[:], lhsT=curT[0][:, b * P:(b + 1) * P],
                                 rhs=wself_t[layer][:], start=False, stop=False)
                nc.tensor.matmul(ph[:], lhsT=onesr_t[:], rhs=crow_t[layer][:],
                                 start=False, stop=True)
                tmp = wp.tile([P, P], BF16, tag="tmp")
                nc.vector.tensor_scalar(tmp[:], ph[:], SLOPE, None, ALU.mult)
                ht = wp.tile([P, P], BF16, tag="ht")
                nc.vector.tensor_tensor(ht[:], ph[:], tmp[:], ALU.max)
                if layer == 0:
                    if b < PART0 // P:
                        nc.sync.dma_start(hshA[b * P:(b + 1) * P, :], ht[:])
                    else:
                        b1 = b - PART0 // P
                        nc.sync.dma_start(hshB[b1 * P:(b1 + 1) * P, :], ht[:])
                pt = ps_t.tile([P, P], BF16, tag="pt")
                nc.tensor.transpose(pt[:], ht[:], identb_t[:])
                nc.scalar.activation(nxtT[0][:, b * P:(b + 1) * P], pt[:],
                                     AF.Copy)
                if layer == 0:
                    if b == NBLK - 1:
                        nc.gpsimd.collective_compute(
                            "AllGather", ALU.bypass, replica_groups=rg,
                            ins=[hshB[:].opt()], outs=[hfull1[:].opt()])
                    if b == PART0 // P - 2:
                        nc.gpsimd.collective_compute(
                            "AllGather", ALU.bypass, replica_groups=rg,
                            ins=[hshA[:].opt()], outs=[hfull0[:].opt()])
                else:
                    pg = ps_l.tile([P, P], F32, tag="ph")
                    nc.tensor.matmul(pg[:, 0:64],
                                     lhsT=nxtT[0][:, b * P:(b + 1) * P],
                                     rhs=amat_t[:], start=True, stop=True)
                    nc.tensor.matmul(pg[:, 64:128],
                                     lhsT=nxtT[0][:, b * P:(b + 1) * P],
                                     rhs=bmat_t[:], start=True, stop=False)
                    nc.tensor.matmul(pg[:, 64:128], lhsT=onesr_t[:],
                                     rhs=mb0r_t[:], start=False, stop=True)
                    gq = wp.tile([P, P], BF16, tag="gq")
                    nc.scalar.activation(gq[:], pg[:], AF.Copy)
                    if b < PART0 // P:
                        nc.sync.dma_start(gqshA[b * P:(b + 1) * P, :], gq[:])
                    else:
                        b1 = b - PART0 // P
                        nc.sync.dma_start(gqshB[b1 * P:(b1 + 1) * P, :], gq[:])
                    if b == PART0 // P - 1:
                        nc.gpsimd.collective_compute(
                            "AllGather", ALU.bypass, replica_groups=rg,
                            ins=[gqshA[:].opt()],
                            outs=[gqfull0[:].opt()])

            def run_gathers(ps, gtab_s, cl, first, mtot):
                done = first
                for (p0, n_idx, ch0) in cl:
                    ncall = n_idx // P
                    g = mp.tile([P, GCALL // P, P], BF16, tag="g")
                    nc.gpsimd.dma_gather(
                        g[:, :ncall, :], gtab_s[:],
                        gidx_t[:, p0 // 16:(p0 + n_idx) // 16],
                        n_idx, n_idx, P, single_packet=False,
                        queue_num=next_q())
                    oht = op_.tile([P, GCALL // P, W], F8, tag="oh")
                    nc.sync.dma_start(
                        oht[:, :ncall, :],
                        ohm[:, ch0 * W:(ch0 + ncall) * W]
                        .rearrange("p (a b) -> p a b", b=W))
                    for cc in range(ncall):
                        nc.tensor.matmul(
                            ps[:], lhsT=g[:, cc, :], rhs=oht[:, cc, :],
                            start=(done == 0), stop=(done == mtot - 1))
                        done += 1

            curT, nxtT = [curT], [nxtT]
            l0_order = list(range(12, NWIN)) + list(range(0, 12))
            for layer in range(2):
                tabs = ((xb[0:T0, :], xb[T0:NTOT, :]) if layer == 0
                        else (hfull0[:], hfull1[:]))
                worder = l0_order if layer == 0 else range(NWIN)
                for w in worder:
                    ww = _win_width(w)
                    wsl = aggrb[:, w * W:w * W + ww]
                    nch0 = int(nch_u[w, 0])
                    nch1 = int(nch_u[w, 1])
                    mtot = nch0 + nch1
                    if mtot == 0:
                        nc.vector.memset(wsl, 0.0)
                    else:
                        ps = ps_w.tile([P, W], F32, tag="psw")
                        if layer == 0:
                            run_gathers(ps, tabs[0], calls[(w, 0)], 0, mtot)
                            run_gathers(ps, tabs[1], calls[(w, 1)], nch0, mtot)
                        else:
                            run_gathers(ps, tabs[1], calls[(w, 1)], 0, mtot)
                            run_gathers(ps, tabs[0], calls[(w, 0)], nch1, mtot)
                        nc.vector.tensor_tensor(
                            wsl, ps[:, :ww],
                            invd_t[:, w * W:w * W + ww], ALU.mult)
                    for b in range((w * W) // P, (w * W + ww) // P):
                        linear_block(layer, b)
                curT[0], nxtT[0] = nxtT[0], curT[0]
            nc.gpsimd.collective_compute(
                "AllGather", ALU.bypass, replica_groups=rg,
                ins=[gqshB[:].opt()], outs=[gqfull1[:].opt()])

            ctxC.__exit__(None, None, None)
            ctxB.__exit__(None, None, None)
            ctxA.__exit__(None, None, None)
            with (
                tc.tile_pool(name="psc1", bufs=2, space="PSUM") as psc1,
                tc.tile_pool(name="psc2", bufs=2, space="PSUM") as psc2,
                tc.tile_pool(name="ps_c", bufs=2, space="PSUM") as ps_c,
                tc.tile_pool(name="ps_y", bufs=1, space="PSUM") as ps_y,
            ):
                # ---- candidate gathers (slot layout, 4-queue) ----
                ucalls, vcalls = cm["ucalls"], cm["vcalls"]
                uci, vci = [], []
                for i, (s0, n, sh) in enumerate(ucalls):
                    uci += [(i, j) for j in range(n // P)]
                for i, (s0, n, sh) in enumerate(vcalls):
                    vci += [(i, j) for j in range(n // P)]
                gu_tiles = [None] * len(ucalls)
                gv_tiles = [None] * len(vcalls)
                for i in range(max(len(ucalls), len(vcalls))):
                    for (tiles, calls, pool, idx_t) in (
                            (gu_tiles, ucalls, gup, cu_t),
                            (gv_tiles, vcalls, gvp, cv_t)):
                        if i >= len(calls):
                            continue
                        (s0, n_idx, sh) = calls[i]
                        ncall = n_idx // P
                        g = pool.tile([P, GCALL // P, P], BF16, tag="g")
                        nc.gpsimd.dma_gather(
                            g[:, :ncall, :],
                            (gqfull0 if sh == 0 else gqfull1)[:],
                            idx_t[:, s0 // 16:(s0 + n_idx) // 16],
                            n_idx, n_idx, P, single_packet=False,
                            queue_num=next_q())
                        tiles[i] = g
                # ---- candidate MLP ----
                nchunk = CSLOT // P
                z1t = None
                fr = None
                for c in range(nchunk):
                    ui, uo = uci[c]
                    vi, vo = vci[c]
                    if c % 4 == 0:
                        fr = fp_.tile([1, CCH], BF16, tag="fr")
                        f0 = (c // 4) * CCH
                        fw = min(CCH, CSLOT - f0)
                        nc.sync.dma_start(fr[:, :fw], featr[:, f0:f0 + fw])
                    psf = psc1.tile([P, 64], F32, tag="psf")
                    nc.tensor.matmul(psf[:], lhsT=identb_t[:],
                                     rhs=gu_tiles[ui][:, uo, 0:64],
                                     start=True, stop=False)
                    nc.tensor.matmul(psf[:], lhsT=identb_t[:],
                                     rhs=gv_tiles[vi][:, vo, 64:128],
                                     start=False, stop=False)
                    nc.tensor.matmul(
                        psf[:], lhsT=fr[:, (c % 4) * P:(c % 4 + 1) * P],
                        rhs=mw0r_t[:], start=False, stop=True)
                    zs = wp.tile([P, 64], BF16, tag="zs")
                    nc.vector.tensor_scalar(zs[:], psf[:], SLOPE, None, ALU.mult)
                    z1b = wp.tile([P, 64], BF16, tag="z1b")
                    nc.vector.tensor_tensor(z1b[:], psf[:], zs[:], ALU.max)
                    ptz = psc2.tile([64, P], BF16, tag="ptz")
                    nc.tensor.transpose(ptz[:], z1b[:], identb_t[:])
                    if c % 4 == 0:
                        z1t = z1p.tile([64, CCH], BF16, tag="z1t")
                    nc.scalar.activation(z1t[:, (c % 4) * P:(c % 4 + 1) * P],
                                         ptz[:], AF.Copy)
                    if c % 4 == 3 or c == nchunk - 1:
                        g0 = (c // 4) * CCH
                        gw = min(CCH, CSLOT - g0)
                        mr = fp_.tile([1, CCH], BF16, tag="mr")
                        nc.sync.dma_start(mr[:, :gw], maskr[:, g0:g0 + gw])
                        psz = ps_c.tile([64, CCH], F32, tag="psc")
                        nc.tensor.matmul(psz[:, :gw], lhsT=mw1_t[:],
                                         rhs=z1t[:, :gw], start=True, stop=False)
                        nc.tensor.matmul(psz[:, :gw], lhsT=mb1r_t[:],
                                         rhs=onesc_t[:, :gw],
                                         start=False, stop=True)
                        zs2 = wp.tile([64, CCH], BF16, tag="zs2")
                        nc.vector.tensor_scalar(zs2[:, :gw], psz[:, :gw],
                                                SLOPE, None, ALU.mult)
                        z2 = wp.tile([64, CCH], BF16, tag="z2")
                        nc.vector.tensor_tensor(z2[:, :gw], psz[:, :gw],
                                                zs2[:, :gw], ALU.max)
                        py = ps_y.tile([1, CCH], F32, tag="psy")
                        nc.tensor.matmul(py[:, :gw], lhsT=mw2_t[:],
                                         rhs=z2[:, :gw], start=True, stop=False)
                        nc.tensor.matmul(py[:, :gw], lhsT=mb2r_t[:],
                                         rhs=onesc_t[:, :gw],
                                         start=False, stop=True)
                        ym = fp_.tile([1, CCH], F32, tag="ym")
                        nc.vector.tensor_tensor(ym[:, :gw], py[:, :gw],
                                                mr[:, :gw], ALU.add)
                        nc.sync.dma_start(ysh[:, g0:g0 + gw], ym[:, :gw])
            nc.gpsimd.collective_compute(
                "AllGather", ALU.bypass, replica_groups=rg,
                ins=[ysh[:].opt()], outs=[yfull[:].opt()])
            # ---- softmax over yfull viewed as [128, YCOLS] ----
            yf = bp.tile([P, YCOLS], F32, tag="yf")
            nc.sync.dma_start(yf[:], yfull[:].rearrange("a b -> (a b)")
                              .rearrange("(p c) -> p c", p=P))
            nc.sync.dma_start(y_out[:], yf[:])
            rmax = wp.tile([P, 1], F32, tag="rmax")
            nc.vector.tensor_reduce(rmax[:], yf[:], mybir.AxisListType.X, ALU.max)
            gmax = wp.tile([P, 1], F32, tag="gmax")
            nc.gpsimd.partition_all_reduce(gmax[:], rmax[:], P,
                                           bass_isa.ReduceOp.max)
            ngmax = wp.tile([P, 1], F32, tag="ngmax")
            nc.vector.tensor_scalar(ngmax[:], gmax[:], -1.0, None, ALU.mult)
            ef = bp.tile([P, YCOLS], F32, tag="ef")
            se = wp.tile([P, 1], F32, tag="se")
            nc.scalar.activation(ef[:], yf[:], AF.Exp, bias=ngmax[:, 0:1],
                                 accum_out=se[:])
            stot = wp.tile([P, 1], F32, tag="stot")
            nc.gpsimd.partition_all_reduce(stot[:], se[:], P,
                                           bass_isa.ReduceOp.add)
            invs = wp.tile([P, 1], F32, tag="invs")
            nc.vector.reciprocal(invs[:], stot[:])
            pf = bp.tile([P, YCOLS], F32, tag="pf")
            nc.vector.tensor_scalar(pf[:], ef[:], invs[:, 0:1], None, ALU.mult)
            nc.sync.dma_start(p_out[:], pf[:])
    nc.compile()
    return nc


def kernel(x, src, dst, cand_u, cand_v, cand_feat,
           w_self0, w_neigh0, b0, gamma0, beta0, rm0, rv0,
           w_self1, w_neigh1, b1, gamma1, beta1, rm1, rv1,
           mw0, mb0, mw1, mb1, mw2, mb2):
    x = np.asarray(x, np.float32)
    src = np.asarray(src, np.int64)
    dst = np.asarray(dst, np.int64)
    cand_u = np.asarray(cand_u, np.int64)
    cand_v = np.asarray(cand_v, np.int64)
    cand_feat = np.asarray(cand_feat, np.float32)

    deg = np.bincount(dst, minlength=N).astype(np.float32)
    invdeg = 1.0 / np.maximum(deg, 1.0)
    em, edata = _prep_edges(src, dst)
    cm, cdata = _prep_cands(cand_u, cand_v, cand_feat)

    xpad = np.zeros((NTOT, D), np.float32)
    xpad[:N] = x
    invdp = np.zeros(NTOT, np.float32)
    invdp[:N] = invdeg
    # permuted gather table: [part0: 8 x rows 0:3200 | part1: 8 x rows 3200:]
    gids = np.arange(NTOT)
    pp, tix = _node_part(gids)
    xperm = np.zeros((NTOT, D), np.float32)
    xperm[np.where(pp == 0, tix, T0 + tix)] = xpad[gids]

    com = {
        "xb": xperm.astype(BF),
        "identb": np.eye(P, dtype=np.float32).astype(BF),
        "onesr": np.ones((1, P), BF),
        "onesc": np.ones((1, CCH), BF),
    }
    for l, (ws, wn, b, ga, be, rme, rve) in enumerate(
        ((w_self0, w_neigh0, b0, gamma0, beta0, rm0, rv0),
         (w_self1, w_neigh1, b1, gamma1, beta1, rm1, rv1))):
        a = (np.asarray(ga) / np.sqrt(np.asarray(rve) + BN_EPS)).astype(np.float32)
        com[f"wself{l}"] = (np.asarray(ws) * a[None, :]).astype(BF)
        com[f"wneigh{l}"] = (np.asarray(wn) * a[None, :]).astype(BF)
        com[f"crow{l}"] = (a * (np.asarray(b) - np.asarray(rme)) + np.asarray(be)).astype(BF)[None, :]
    com["amat"] = np.asarray(mw0[0:128], np.float32).astype(BF)
    com["bmat"] = np.asarray(mw0[128:256], np.float32).astype(BF)
    com["mw0r"] = np.asarray(mw0[256], np.float32).astype(BF)[None, :]
    com["mb0r"] = np.asarray(mb0, np.float32).astype(BF)[None, :]
    com["mw1"] = np.asarray(mw1, np.float32).astype(BF)
    com["mb1r"] = np.asarray(mb1, np.float32).astype(BF)[None, :]
    com["mw2c"] = np.asarray(mw2, np.float32).astype(BF)
    com["mb2r"] = np.asarray(mb2, np.float32).reshape(1, 1).astype(BF)

    nc = _build_nc(em, cm)
    in_maps = []
    for k in range(NCORE):
        m = dict(com)
        m["xT"] = xpad[k * NSH:(k + 1) * NSH].T.astype(BF).copy()
        m["invd"] = np.tile(invdp[k * NSH:(k + 1) * NSH].astype(BF), (P, 1))
        m["gidx"] = edata[k]["gidx"]
        m["ohm"] = edata[k]["ohm"]
        m["cu"] = cdata[k]["cu"]
        m["cv"] = cdata[k]["cv"]
        m["featr"] = cdata[k]["feat"]
        m["maskr"] = cdata[k]["mask"]
        in_maps.append(m)
    trace = bool(os.environ.get("KERNEL_TRACE"))
    if trace:
        import types
        if "antenv.axon_hooks" not in sys.modules:
            try:
                import antenv
                from trn_agent_boot.trn_boot import _ntff_profile_via_ctypes
                mod = types.ModuleType("antenv.axon_hooks")
                hook = [_ntff_profile_via_ctypes("/opt/axon/libaxon_pjrt.so")]
                mod.set_axon_ntff_profile_hook = lambda h: hook.__setitem__(0, h)
                mod.get_axon_ntff_profile_hook = lambda: hook[0]
                sys.modules["antenv.axon_hooks"] = mod
                antenv.axon_hooks = mod
            except Exception:
                trace = False
    res = run_bass_kernel_spmd(nc, in_maps, core_ids=list(range(NCORE)),
                               trace=trace,
                               tmpdir=os.environ.get("KERNEL_TRACE_DIR"))
    if trace and res.exec_time_ns is not None:
        print(f"HW exec time: {res.exec_time_ns} ns")
    y_all = np.zeros(C, np.float32)
    p_all = np.zeros(C, np.float32)
    cslot = cm["cslot"]
    y_lin = res.results[0]["y_out"].ravel()   # flat order: core, slot
    p_lin = res.results[0]["p_out"].ravel()
    for k in range(NCORE):
        sm = cdata[k]["slotmap"]
        valid = sm >= 0
        j = np.nonzero(valid)[0]
        y_all[sm[valid]] = y_lin[k * cslot + j]
        p_all[sm[valid]] = p_lin[k * cslot + j]
    return y_all[:, None], p_all[:, None]
